# revision 22
# baseline (speedup 1.0000x reference)
"""Trainium2 Bass kernel for a GPT-2 style transformer block (nn_Block_16690242913196).

Sharding (8 NeuronCores, identical SPMD program):
  - LN1/QKV/proj/LN2/MLP: token-parallel (core i owns 512 flat tokens).
  - Attention: head-parallel (core i owns heads {2i, 2i+1}, all tokens).
  - Collective 1: AllGather of the RAW bf16 input (1MB/rank), triggered
    at t~0 straight from DRAM — no LN dependency, so it absorbs the
    cross-core start skew under useful work.
  - Collective 2: tiny AllGather of LN1 stats rows (mu, rstd; 2KB).
    Each consumer normalizes rank r's activations itself (2 broadcast
    matmuls + 16 bf16 DVE ops per rank, hidden in the QKV/attention era).
  - Collective 3: AllToAll of UNnormalized attention outputs plus
    RECIPROCAL softmax denominators (130 rows x 512 bf16); the divide
    becomes a broadcast-matmul + multiply on the token-parallel side.

  LN gamma/beta are folded into the consuming weights host-side. rstd
  is computed as exp(-0.5*ln(var+eps)) so LNs and the attention exp
  share one ACT table set. LN2 stats accumulate inside the proj loop.
"""

import numpy as np
import ml_dtypes

P = 128
B, S, D, H = 2, 2048, 1024, 16
DH = D // H          # 64
DI = 4 * D           # 4096
EPS = 1e-5
NCORES = 8
TT = B * S           # 4096 flat tokens
TOK = TT // NCORES   # 512 tokens per core
KD = D // P          # 8
KDI = DI // P        # 32
QCH = 256            # query chunk (2 blocks of 128)
NQC = S // QCH       # 8 query chunks per batch
HL = H // NCORES     # 2 local heads
RG = [list(range(NCORES))]

_CACHED_NC = None


def build_nc():
    import concourse.bacc as bacc
    import concourse.tile as tile
    import concourse.mybir as mybir
    from contextlib import ExitStack

    dt = mybir.dt
    f32, bf16, f32r = dt.float32, dt.bfloat16, dt.float32r
    f8 = dt.float8e4
    DR = mybir.MatmulPerfMode.DoubleRow
    AF = mybir.ActivationFunctionType
    OP = mybir.AluOpType

    nc = bacc.Bacc("TRN2", target_bir_lowering=False, debug=False,
                   num_devices=NCORES)

    # ---- kernel I/O (per-core shapes) ----
    xT = nc.dram_tensor("xT", [P, KD, TOK], bf16, kind="ExternalInput").ap()
    aw = nc.dram_tensor("aw", [P, KD, 3 * P], bf16, kind="ExternalInput").ap()
    ab = nc.dram_tensor("ab", [3 * P], f32, kind="ExternalInput").ap()
    pw = nc.dram_tensor("pw", [KD, P, KD, P], bf16, kind="ExternalInput").ap()
    pb = nc.dram_tensor("pb", [P, KD], f32, kind="ExternalInput").ap()
    fw = nc.dram_tensor("fw", [KDI, P, KD, P], bf16, kind="ExternalInput").ap()
    fb = nc.dram_tensor("fb", [P, KDI], f32, kind="ExternalInput").ap()
    gw = nc.dram_tensor("gw", [KD, P, KDI, P], bf16, kind="ExternalInput").ap()
    gb = nc.dram_tensor("gb", [P, KD], f32, kind="ExternalInput").ap()
    mk = nc.dram_tensor("mk", [2, P, QCH], bf16, kind="ExternalInput").ap()
    outT = nc.dram_tensor("outT", [D, TOK], f32, kind="ExternalOutput").ap()

    with tile.TileContext(nc) as tc, ExitStack() as ctx:
        const = ctx.enter_context(tc.tile_pool(name="const", bufs=1))
        dram = ctx.enter_context(tc.tile_pool(name="dram", bufs=1, space="DRAM"))
        psum = ctx.enter_context(tc.tile_pool(name="psum", bufs=1, space="PSUM"))
        rows = ctx.enter_context(tc.tile_pool(name="rows", bufs=6))
        sqp = ctx.enter_context(tc.tile_pool(name="sqp", bufs=2))
        lnt = ctx.enter_context(tc.tile_pool(name="lnt", bufs=3))
        dfp = ctx.enter_context(tc.tile_pool(name="dfp", bufs=8))
        res = ctx.enter_context(tc.tile_pool(name="res", bufs=1))

        # collective bounce buffers
        cc1_in = dram.tile([P, KD, TOK], bf16)
        cc1_out = dram.tile([NCORES, P, KD, TOK], bf16, addr_space="Shared")
        cc2_in = dram.tile([NCORES, P + 2, TOK], bf16)
        cc2_out = dram.tile([NCORES, P + 2, TOK], bf16)

        # input activations into SBUF (split per-k across DMA queues)
        xT_sb = res.tile([P, KD, TOK], bf16)
        for k in range(KD):
            nc.sync.dma_start(xT_sb[:, k, :], xT[:, k, :])

        # ---- constants in SBUF ----
        aw_sb = const.tile([P, KD, 3 * P], bf16)
        nc.sync.dma_start(aw_sb, aw)
        qbias = const.tile([P, 1], f32)
        nc.sync.dma_start(qbias, ab[0:P].rearrange("(p o) -> p o", o=1))
        kbias = const.tile([P, 1], f32)
        nc.sync.dma_start(kbias, ab[P:2 * P].rearrange("(p o) -> p o", o=1))
        vb_bc = const.tile([P, P], f32)
        nc.sync.dma_start(vb_bc, ab[2 * P:3 * P].rearrange("(o c) -> o c", o=1).to_broadcast((P, P)))
        mkc = const.tile([P, 2, QCH], bf16)
        nc.sync.dma_start(mkc, mk.rearrange("m p q -> p m q"))
        pb_sb = const.tile([P, KD], f32)
        nc.sync.dma_start(pb_sb, pb)
        fb_sb = const.tile([P, KDI], f32)
        nc.sync.dma_start(fb_sb, fb)
        gb_sb = const.tile([P, KD], f32)
        nc.sync.dma_start(gb_sb, gb)
        ones_cf = const.tile([P, 1], f32)
        nc.vector.memset(ones_cf, 1.0)
        ones_c = const.tile([P, 1], f32r)
        nc.vector.tensor_copy(ones_c, ones_cf)
        ones_cb = const.tile([P, 1], bf16)
        nc.vector.tensor_copy(ones_cb, ones_cf)
        ones_rf = const.tile([1, P], f32)
        nc.vector.memset(ones_rf, 1.0)
        ones_r = const.tile([1, P], f32r)
        nc.vector.tensor_copy(ones_r, ones_rf)
        ones_rb = const.tile([1, P], bf16)
        nc.vector.tensor_copy(ones_rb, ones_rf)
        eps_sb = const.tile([1, 1], f32)
        nc.vector.memset(eps_sb, EPS)

        # phase-scoped pools: attention-era tiles are freed before the MLP
        # pools (notably the 4MB hT) allocate.
        s1 = ExitStack()
        ares = s1.enter_context(tc.tile_pool(name="ares", bufs=1))
        xnp = s1.enter_context(tc.tile_pool(name="xnp", bufs=2))
        wp = s1.enter_context(tc.tile_pool(name="wp", bufs=4))
        psA = s1.enter_context(tc.tile_pool(name="psA", bufs=2, space="PSUM"))

        # ---- phase 1: LN1 stats only (full width) -> tiny stats AG ----
        with nc.named_scope("ln1"):
            sx = psum.tile([1, TOK], f32, tag="sm1", bufs=1, name="l1sx")
            for k in range(KD):
                nc.tensor.matmul(sx, ones_cb, xT_sb[:, k, :],
                                 start=(k == 0), stop=(k == KD - 1))
            sxx = psum.tile([1, TOK], f32, tag="sm2", bufs=1, name="l1sxx")
            for k in range(KD):
                sq = sqp.tile([P, TOK], bf16, tag="sq")
                nc.vector.tensor_mul(sq, xT_sb[:, k, :], xT_sb[:, k, :])
                nc.tensor.matmul(sxx, ones_cb, sq,
                                 start=(k == 0), stop=(k == KD - 1))
            muf = rows.tile([1, TOK], f32, tag="row")
            nc.vector.tensor_scalar_mul(muf, sx, 1.0 / D)
            m2 = rows.tile([1, TOK], f32, tag="row")
            nc.vector.tensor_scalar_mul(m2, sxx, 1.0 / D)
            var = rows.tile([1, TOK], f32, tag="row")
            nc.vector.tensor_tensor(out=var, in0=muf, in1=muf, op=OP.mult)
            nc.vector.tensor_tensor(out=var, in0=m2, in1=var, op=OP.subtract)
            lnv = rows.tile([1, TOK], f32, tag="row")
            nc.scalar.activation(lnv, var, AF.Ln, bias=eps_sb[:])
            rstd = rows.tile([1, TOK], bf16, tag="rowb")
            with nc.allow_low_precision(reason="bf16 rstd broadcast"):
                nc.scalar.activation(rstd, lnv, AF.Exp, scale=-0.5)
            mub16 = rows.tile([1, TOK], bf16, tag="rowb")
            with nc.allow_low_precision(reason="bf16 mu broadcast"):
                nc.vector.tensor_copy(mub16, muf)
            mub = psum.tile([P, TOK], f32, tag="big", bufs=2, name="l1mub")
            nc.tensor.matmul(mub, ones_rb, mub16, start=True, stop=True)
            rsb = psum.tile([P, TOK], f32, tag="big", bufs=2, name="l1rsb")
            nc.tensor.matmul(rsb, ones_rb, rstd, start=True, stop=True)
            xn = res.tile([P, KD, TOK], bf16)
            for k in range(KD):
                t1 = lnt.tile([P, TOK], bf16, tag="ln1")
                nc.vector.tensor_tensor(out=t1, in0=xT_sb[:, k, :], in1=mub,
                                        op=OP.subtract)
                nc.vector.tensor_tensor(out=xn[:, k, :], in0=t1, in1=rsb,
                                        op=OP.mult)
                nc.sync.dma_start(cc1_in[:, k, :], xn[:, k, :])
        with nc.named_scope("agx"):
            nc.gpsimd.collective_compute(
                "AllGather", OP.bypass, replica_groups=RG,
                ins=[cc1_in[:].opt()], outs=[cc1_out[:].opt()])

        # ---- phase 2+3: QKV (per rank) interleaved with attention ----
        kT = [ares.tile([P, TOK], bf16, name=f"kT{r}") for r in range(NCORES)]
        vA = [ares.tile([P, 4, HL, DH + 8], f8, name=f"vA{r}")
              for r in range(NCORES)]
        qT = [ares.tile([P, TOK], bf16, name=f"qT{r}") for r in range(NCORES)]
        for r in range(NCORES):
            nc.vector.memset(vA[r][:, :, :, DH:DH + 1], 1.0)
        aT = ares.tile([P, NCORES, TOK], bf16)
        dT = ares.tile([1, HL, NCORES, TOK], bf16)

        def qkv_unit(r):
            with nc.named_scope(f"qkv{r}"):
                xn_r = xnp.tile([P, KD, TOK], bf16, tag="xnr")
                for kh in range(2):
                    nc.sync.dma_start(xn_r[:, 4 * kh:4 * (kh + 1), :],
                                      cc1_out[r][:, 4 * kh:4 * (kh + 1), :])
                for which in range(2):  # 0 -> q, 1 -> k
                    ps = psum.tile([P, TOK], f32, tag="big", bufs=2)
                    cb = which * P
                    for k in range(KD):
                        nc.tensor.matmul(ps, aw_sb[:, k, cb:cb + P],
                                         xn_r[:, k, :],
                                         start=(k == 0), stop=(k == KD - 1))
                    dst = qT[r] if which == 0 else kT[r]
                    bias = qbias if which == 0 else kbias
                    nc.scalar.activation(dst, ps, AF.Identity, bias=bias[:])
                for t in range(4):
                    psv = psum.tile([P, P], f32, tag="sm1", bufs=1)
                    for k in range(KD):
                        nc.tensor.matmul(psv, xn_r[:, k, P * t:P * (t + 1)],
                                         aw_sb[:, k, 2 * P:3 * P],
                                         start=(k == 0), stop=(k == KD - 1))
                    nc.vector.tensor_tensor(
                        out=vA[r][:, t, :, 0:DH],
                        in0=psv.rearrange("p (h d) -> p h d", h=HL),
                        in1=vb_bc.rearrange("p (h d) -> p h d", h=HL),
                        op=OP.add)

        def attn_unit(b, qc):
            """Causal attention chunk: UNnormalized numerators into aT,
            reciprocal softmax denominators into dT (divide post-A2A).
            Software-pipelined: scores(pair p+1) overlap exp(pair p)."""
            qr = 4 * b + qc // 2
            qo = QCH * (qc % 2)
            nkb = 2 * qc + 2
            npair = nkb // 2
            accs = [psA.tile([DH + 1, QCH], f32, tag="acc", bufs=2,
                             name=f"acc{b}_{qc}_{h}")
                    for h in range(HL)]

            def emit_scores(p):
                kb0 = 2 * p
                ws = []
                for h in range(HL):
                    hb = DH * h
                    sc = psA.tile([P, 2 * QCH], f32, tag="sc", bufs=2)
                    for j in range(2):
                        kb = kb0 + j
                        r = 4 * b + kb // 4
                        t = kb % 4
                        nc.tensor.matmul(
                            sc[:, QCH * j:QCH * (j + 1)],
                            kT[r][hb:hb + DH, P * t:P * (t + 1)],
                            qT[qr][hb:hb + DH, qo:qo + QCH],
                            start=True, stop=True,
                            skip_group_check=True)
                    w = wp.tile([P, 2 * QCH], f8, tag="w")
                    with nc.allow_low_precision(reason="fp8 softmax weights"):
                        nc.scalar.activation(w, sc, AF.Exp, scale=0.125)
                    if kb0 == 2 * qc:  # diagonal pair: apply masks
                        nc.vector.tensor_mul(
                            w.rearrange("p (m q) -> p m q", m=2),
                            w.rearrange("p (m q) -> p m q", m=2),
                            mkc)
                    ws.append(w)
                return ws

            def emit_av(p, ws):
                kb0 = 2 * p
                r = 4 * b + kb0 // 4
                t0 = kb0 % 4
                for h in range(HL):
                    nc.tensor.matmul(
                        accs[h], vA[r][:, t0:t0 + 2, h, 0:DH + 1],
                        ws[h].rearrange("p (two q) -> p two q", two=2),
                        start=(p == 0), stop=(p == npair - 1),
                        skip_group_check=True, perf_mode=DR)

            ws_p = emit_scores(0)
            for p in range(npair):
                ws_n = emit_scores(p + 1) if p + 1 < npair else None
                emit_av(p, ws_p)
                ws_p = ws_n
            for h in range(HL):
                hb = DH * h
                nc.vector.tensor_copy(aT[hb:hb + DH, qr, qo:qo + QCH],
                                      accs[h][0:DH, :])
                nc.vector.tensor_copy(dT[0:1, h, qr, qo:qo + QCH],
                                      accs[h][DH:DH + 1, :])
            if qc % 2 == 1:  # chunk qr complete: stage its A2A slot early
                nc.sync.dma_start(cc2_in[qr, 0:P], aT[:, qr, :])
                nc.sync.dma_start(cc2_in[qr, P:P + 2], dT[0:1, :, qr, :])

        with nc.named_scope("qkv_attn"):
            qkv_unit(0)
            for b in range(B):
                for qc in range(NQC):
                    attn_unit(b, qc)
                    if qc % 2 == 0:
                        r = 4 * b + qc // 2 + 1
                        if r < NCORES:
                            qkv_unit(r)

        # ---- phase 4: AllToAll back to token-parallel ----
        with nc.named_scope("a2a"):
            nc.gpsimd.collective_compute(
                "AllToAll", OP.bypass, replica_groups=RG,
                ins=[cc2_in[:].opt()], outs=[cc2_out[:].opt()])
        # latency-critical result loads: emit BEFORE the MLP weight
        # avalanche (s1.close frees SBUF and unleashes wgt prefetch DMAs)
        aF = [dfp.tile([P, TOK], bf16, tag="aF", name=f"aF{r}")
              for r in range(NCORES)]
        aFn = [dfp.tile([P, TOK], bf16, tag="aFn", name=f"aFn{r}")
               for r in range(NCORES)]
        dfh = [dfp.tile([1, 2 * TOK], bf16, tag="dfh", name=f"dfh{r}")
               for r in range(NCORES)]
        with nc.named_scope("resload"):
            for r in range(NCORES):
                nc.sync.dma_start(aF[r], cc2_out[r, 0:P])
                nc.sync.dma_start(dfh[r][:, 0:TOK], cc2_out[r, P:P + 1])
                nc.sync.dma_start(dfh[r][:, TOK:2 * TOK],
                                  cc2_out[r, P + 1:P + 2])
            rcps = []
            for r in range(NCORES):
                rcp = dfp.tile([1, 2 * TOK], bf16, tag="rcp", name=f"rcp{r}")
                with nc.allow_low_precision(reason="bf16 softmax denom"):
                    nc.vector.reciprocal(rcp, dfh[r])
                rcps.append(rcp)

        s1.close()  # release attention-era SBUF
        psB = ctx.enter_context(tc.tile_pool(name="psB", bufs=4, space="PSUM"))
        mlp = ctx.enter_context(tc.tile_pool(name="mlp", bufs=1))
        wgt = ctx.enter_context(tc.tile_pool(name="wgt", bufs=1))
        outp = ctx.enter_context(tc.tile_pool(name="outp", bufs=2))

        # ---- phase 5: softmax normalize + output projection + residual ----
        h1T = mlp.tile([P, KD, TOK], bf16)
        with nc.named_scope("proj"):
            for r in range(NCORES):
                rcp = rcps[r]
                rb0 = psB.tile([DH, TOK], f32, tag="rb",
                               bufs=4, name=f"rb0_{r}")
                nc.tensor.matmul(rb0, ones_rb[:, 0:DH], rcp[:, 0:TOK],
                                 start=True, stop=True, skip_group_check=True)
                rb1 = psB.tile([DH, TOK], f32, tag="rb",
                               bufs=4, name=f"rb1_{r}")
                nc.tensor.matmul(rb1, ones_rb[:, 0:DH], rcp[:, TOK:],
                                 start=True, stop=True, skip_group_check=True)
                nc.vector.tensor_mul(aFn[r][0:DH, :], aF[r][0:DH, :], rb0)
                nc.vector.tensor_mul(aFn[r][DH:P, :], aF[r][DH:P, :], rb1)
            # proj matmuls + incremental LN2 stats (chained across f)
            sx2 = psum.tile([1, TOK], f32, tag="sm1", bufs=1, name="l2sx")
            sxx2 = psum.tile([1, TOK], f32, tag="sm2", bufs=1, name="l2sxx")
            for f in range(KD):
                pwt = wgt.tile([P, KD, P], bf16, tag="pw", bufs=2)
                nc.sync.dma_start(pwt, pw[f])
                ps = psum.tile([P, TOK], f32, tag="big", bufs=2)
                for k in range(KD):
                    nc.tensor.matmul(ps, pwt[:, k, :], aFn[k],
                                     start=(k == 0), stop=(k == KD - 1))
                t1 = lnt.tile([P, TOK], f32, tag="pj")
                nc.vector.tensor_scalar_add(t1, ps, pb_sb[:, f:f + 1])
                nc.vector.tensor_tensor(out=h1T[:, f, :], in0=t1,
                                        in1=xT_sb[:, f, :], op=OP.add)
                nc.tensor.matmul(sx2, ones_cb, h1T[:, f, :],
                                 start=(f == 0), stop=(f == KD - 1))
                sq2 = sqp.tile([P, TOK], bf16, tag="sq2")
                nc.vector.tensor_mul(sq2, h1T[:, f, :], h1T[:, f, :])
                nc.tensor.matmul(sxx2, ones_cb, sq2,
                                 start=(f == 0), stop=(f == KD - 1))

        # ---- phase 6: LN2 finish (stats already accumulated) ----
        mT = mlp.tile([P, KD, TOK], bf16)
        with nc.named_scope("ln2"):
            muf = rows.tile([1, TOK], f32, tag="row")
            nc.vector.tensor_scalar_mul(muf, sx2, 1.0 / D)
            m2 = rows.tile([1, TOK], f32, tag="row")
            nc.vector.tensor_scalar_mul(m2, sxx2, 1.0 / D)
            var = rows.tile([1, TOK], f32, tag="row")
            nc.vector.tensor_tensor(out=var, in0=muf, in1=muf, op=OP.mult)
            nc.vector.tensor_tensor(out=var, in0=m2, in1=var, op=OP.subtract)
            lnv = rows.tile([1, TOK], f32, tag="row")
            nc.scalar.activation(lnv, var, AF.Ln, bias=eps_sb[:])
            rstd = rows.tile([1, TOK], bf16, tag="rowb")
            with nc.allow_low_precision(reason="bf16 rstd broadcast"):
                nc.scalar.activation(rstd, lnv, AF.Exp, scale=-0.5)
            mub16 = rows.tile([1, TOK], bf16, tag="rowb")
            with nc.allow_low_precision(reason="bf16 mu broadcast"):
                nc.vector.tensor_copy(mub16, muf)
            mub = psum.tile([P, TOK], f32, tag="big", bufs=2, name="l2mub")
            nc.tensor.matmul(mub, ones_rb, mub16, start=True, stop=True)
            rsb = psum.tile([P, TOK], f32, tag="big", bufs=2, name="l2rsb")
            nc.tensor.matmul(rsb, ones_rb, rstd, start=True, stop=True)
            for k in range(KD):
                t1 = lnt.tile([P, TOK], bf16, tag="ln2")
                nc.vector.tensor_tensor(out=t1, in0=h1T[:, k, :], in1=mub,
                                        op=OP.subtract)
                nc.vector.tensor_tensor(out=mT[:, k, :], in0=t1, in1=rsb,
                                        op=OP.mult)

        # ---- phase 7: MLP ----
        hT = mlp.tile([P, KDI, TOK], bf16)
        with nc.named_scope("fc1"):
            for j in range(KDI):
                fwt = wgt.tile([P, KD, P], bf16, tag="fw", bufs=3)
                nc.sync.dma_start(fwt, fw[j])
                ps = psum.tile([P, TOK], f32, tag="big", bufs=2)
                for k in range(KD):
                    nc.tensor.matmul(ps, fwt[:, k, :], mT[:, k, :],
                                     start=(k == 0), stop=(k == KD - 1))
                nc.scalar.activation(hT[:, j, :], ps, AF.Gelu_apprx_tanh,
                                     bias=fb_sb[:, j:j + 1])
        with nc.named_scope("fc2"):
            for f in range(KD):
                gwt = wgt.tile([P, KDI, P], bf16, tag="gw", bufs=2)
                nc.sync.dma_start(gwt, gw[f])
                ps = psum.tile([P, TOK], f32, tag="big", bufs=2)
                for k in range(KDI):
                    nc.tensor.matmul(ps, gwt[:, k, :], hT[:, k, :],
                                     start=(k == 0), stop=(k == KDI - 1))
                o = outp.tile([P, TOK], f32, tag="ot")
                nc.vector.tensor_scalar_add(o, ps, gb_sb[:, f:f + 1])
                nc.vector.tensor_tensor(out=o, in0=o, in1=h1T[:, f, :],
                                        op=OP.add)
                nc.sync.dma_start(outT[P * f:P * (f + 1), :], o)

    nc.compile()
    return nc


def shard_inputs(inputs):
    """Full inputs -> list of 8 per-core input dicts (host-side layout only)."""
    bf16 = ml_dtypes.bfloat16
    f32 = np.float32
    hs = np.asarray(inputs["hidden_states"], f32).reshape(TT, D)
    l1g = np.asarray(inputs["ln1_g"], f32)
    l1b = np.asarray(inputs["ln1_b"], f32)
    l2g = np.asarray(inputs["ln2_g"], f32)
    l2b = np.asarray(inputs["ln2_b"], f32)
    # fold LN1 gamma/beta into attn_w/attn_b, LN2 into fc_w/fc_b
    attn_w = np.asarray(inputs["attn_w"], f32) * l1g[:, None]
    attn_b = np.asarray(inputs["attn_b"], f32) + l1b @ np.asarray(inputs["attn_w"], f32)
    fc_w = np.asarray(inputs["fc_w"], f32) * l2g[:, None]
    fc_b = np.asarray(inputs["fc_b"], f32) + l2b @ np.asarray(inputs["fc_w"], f32)

    def col(v):  # [D] -> [P, KD]
        return np.ascontiguousarray(np.asarray(v, f32).reshape(KD, P).T)

    pw = np.ascontiguousarray(np.asarray(inputs["proj_w"], f32)
                              .reshape(KD, P, KD, P).transpose(2, 1, 0, 3)
                              .astype(bf16))
    fw = np.ascontiguousarray(fc_w.reshape(KD, P, KDI, P).transpose(2, 1, 0, 3)
                              .astype(bf16))
    gw = np.ascontiguousarray(np.asarray(inputs["fc2_w"], f32)
                              .reshape(KDI, P, KD, P).transpose(2, 1, 0, 3)
                              .astype(bf16))
    pb = col(inputs["proj_b"])
    fbv = np.ascontiguousarray(fc_b.reshape(KDI, P).T)
    gbv = col(inputs["fc2_b"])

    ii, jj = np.meshgrid(np.arange(P), np.arange(QCH), indexing="ij")
    mkv = np.stack([(jj >= ii), (jj >= ii + P)]).astype(bf16)

    maps = []
    for c in range(NCORES):
        cols = np.r_[P * c:P * (c + 1),
                     D + P * c:D + P * (c + 1),
                     2 * D + P * c:2 * D + P * (c + 1)]
        aw_c = np.ascontiguousarray(
            attn_w[:, cols].reshape(KD, P, 3 * P).transpose(1, 0, 2)
            .astype(bf16))
        ab_c = np.ascontiguousarray(attn_b[cols], dtype=f32)
        xT_c = np.ascontiguousarray(
            hs[TOK * c:TOK * (c + 1)].T.reshape(KD, P, TOK).transpose(1, 0, 2)
        ).astype(bf16)
        maps.append({
            "xT": xT_c, "aw": aw_c, "ab": ab_c,
            "pw": pw, "pb": pb, "fw": fw, "fb": fbv, "gw": gw, "gb": gbv,
            "mk": mkv,
        })
    return maps


def unshard(results):
    out = np.concatenate([np.asarray(r["outT"]).T for r in results], axis=0)
    return np.ascontiguousarray(out.reshape(B, S, D))


def kernel(**inputs):
    global _CACHED_NC
    from concourse.bass_utils import run_bass_kernel_spmd
    if _CACHED_NC is None:
        _CACHED_NC = build_nc()
    in_maps = shard_inputs(inputs)
    res = run_bass_kernel_spmd(_CACHED_NC, in_maps,
                               core_ids=list(range(NCORES)))
    return unshard(res.results)


# revision 23
# speedup vs baseline: 1.0165x; 1.0165x over previous
"""Trainium2 Bass kernel for a GPT-2 style transformer block (nn_Block_16690242913196).

Sharding (8 NeuronCores, identical SPMD program):
  - LN1/QKV/proj/LN2/MLP: token-parallel (core i owns 512 flat tokens).
  - Attention: head-parallel (core i owns heads {2i, 2i+1}, all tokens).
  - Collective 1: AllGather of the RAW bf16 input (1MB/rank), triggered
    at t~0 straight from DRAM — no LN dependency, so it absorbs the
    cross-core start skew under useful work.
  - Collective 2: tiny AllGather of LN1 stats rows (mu, rstd; 2KB).
    Each consumer normalizes rank r's activations itself (2 broadcast
    matmuls + 16 bf16 DVE ops per rank, hidden in the QKV/attention era).
  - Collective 3: AllToAll of UNnormalized attention outputs plus
    RECIPROCAL softmax denominators (130 rows x 512 bf16); the divide
    becomes a broadcast-matmul + multiply on the token-parallel side.

  LN gamma/beta are folded into the consuming weights host-side. rstd
  is computed as exp(-0.5*ln(var+eps)) so LNs and the attention exp
  share one ACT table set. LN2 stats accumulate inside the proj loop.
"""

import numpy as np
import ml_dtypes

P = 128
B, S, D, H = 2, 2048, 1024, 16
DH = D // H          # 64
DI = 4 * D           # 4096
EPS = 1e-5
NCORES = 8
TT = B * S           # 4096 flat tokens
TOK = TT // NCORES   # 512 tokens per core
KD = D // P          # 8
KDI = DI // P        # 32
QCH = 256            # query chunk (2 blocks of 128)
NQC = S // QCH       # 8 query chunks per batch
HL = H // NCORES     # 2 local heads
RG = [list(range(NCORES))]

_CACHED_NC = None


def build_nc():
    import concourse.bacc as bacc
    import concourse.tile as tile
    import concourse.mybir as mybir
    from contextlib import ExitStack

    dt = mybir.dt
    f32, bf16, f32r = dt.float32, dt.bfloat16, dt.float32r
    f8 = dt.float8e4
    DR = mybir.MatmulPerfMode.DoubleRow
    AF = mybir.ActivationFunctionType
    OP = mybir.AluOpType

    nc = bacc.Bacc("TRN2", target_bir_lowering=False, debug=False,
                   num_devices=NCORES)

    # ---- kernel I/O (per-core shapes) ----
    xT = nc.dram_tensor("xT", [P, KD, TOK], bf16, kind="ExternalInput").ap()
    aw = nc.dram_tensor("aw", [P, KD, 3 * P], bf16, kind="ExternalInput").ap()
    ab = nc.dram_tensor("ab", [3 * P], f32, kind="ExternalInput").ap()
    pw = nc.dram_tensor("pw", [KD, P, KD, P], bf16, kind="ExternalInput").ap()
    pb = nc.dram_tensor("pb", [P, KD], f32, kind="ExternalInput").ap()
    fw = nc.dram_tensor("fw", [KDI, P, KD, P], bf16, kind="ExternalInput").ap()
    fb = nc.dram_tensor("fb", [P, KDI], f32, kind="ExternalInput").ap()
    gw = nc.dram_tensor("gw", [KD, P, KDI, P], bf16, kind="ExternalInput").ap()
    gb = nc.dram_tensor("gb", [P, KD], f32, kind="ExternalInput").ap()
    mk = nc.dram_tensor("mk", [2, P, QCH], bf16, kind="ExternalInput").ap()
    outT = nc.dram_tensor("outT", [D, TOK], f32, kind="ExternalOutput").ap()

    with tile.TileContext(nc) as tc, ExitStack() as ctx:
        const = ctx.enter_context(tc.tile_pool(name="const", bufs=1))
        dram = ctx.enter_context(tc.tile_pool(name="dram", bufs=1, space="DRAM"))
        psum = ctx.enter_context(tc.tile_pool(name="psum", bufs=1, space="PSUM"))
        rows = ctx.enter_context(tc.tile_pool(name="rows", bufs=6))
        sqp = ctx.enter_context(tc.tile_pool(name="sqp", bufs=2))
        lnt = ctx.enter_context(tc.tile_pool(name="lnt", bufs=3))
        dfp = ctx.enter_context(tc.tile_pool(name="dfp", bufs=8))
        res = ctx.enter_context(tc.tile_pool(name="res", bufs=1))

        # collective bounce buffers
        cc1_in = dram.tile([P, KD, TOK], bf16)
        cc1_out = dram.tile([NCORES, P, KD, TOK], bf16, addr_space="Shared")
        cc2_in = dram.tile([NCORES, P + 2, TOK], bf16)
        cc2_out = dram.tile([NCORES, P + 2, TOK], bf16)

        # input activations into SBUF (split per-k across DMA queues)
        xT_sb = res.tile([P, KD, TOK], bf16)
        for k in range(KD):
            nc.sync.dma_start(xT_sb[:, k, :], xT[:, k, :])

        # ---- constants in SBUF ----
        aw_sb = const.tile([P, KD, 3 * P], bf16)
        nc.sync.dma_start(aw_sb, aw)
        qbias = const.tile([P, 1], f32)
        nc.sync.dma_start(qbias, ab[0:P].rearrange("(p o) -> p o", o=1))
        kbias = const.tile([P, 1], f32)
        nc.sync.dma_start(kbias, ab[P:2 * P].rearrange("(p o) -> p o", o=1))
        vb_bc = const.tile([P, P], f32)
        nc.sync.dma_start(vb_bc, ab[2 * P:3 * P].rearrange("(o c) -> o c", o=1).to_broadcast((P, P)))
        mkc = const.tile([P, 2, QCH], bf16)
        nc.sync.dma_start(mkc, mk.rearrange("m p q -> p m q"))
        pb_sb = const.tile([P, KD], f32)
        nc.sync.dma_start(pb_sb, pb)
        fb_sb = const.tile([P, KDI], f32)
        nc.sync.dma_start(fb_sb, fb)
        gb_sb = const.tile([P, KD], f32)
        nc.sync.dma_start(gb_sb, gb)
        ones_cf = const.tile([P, 1], f32)
        nc.vector.memset(ones_cf, 1.0)
        ones_c = const.tile([P, 1], f32r)
        nc.vector.tensor_copy(ones_c, ones_cf)
        ones_cb = const.tile([P, 1], bf16)
        nc.vector.tensor_copy(ones_cb, ones_cf)
        ones_rf = const.tile([1, P], f32)
        nc.vector.memset(ones_rf, 1.0)
        ones_r = const.tile([1, P], f32r)
        nc.vector.tensor_copy(ones_r, ones_rf)
        ones_rb = const.tile([1, P], bf16)
        nc.vector.tensor_copy(ones_rb, ones_rf)
        eps_sb = const.tile([1, 1], f32)
        nc.vector.memset(eps_sb, EPS)

        # phase-scoped pools: attention-era tiles are freed before the MLP
        # pools (notably the 4MB hT) allocate.
        s1 = ExitStack()
        ares = s1.enter_context(tc.tile_pool(name="ares", bufs=1))
        xnp = s1.enter_context(tc.tile_pool(name="xnp", bufs=2))
        wp = s1.enter_context(tc.tile_pool(name="wp", bufs=4))
        psA = s1.enter_context(tc.tile_pool(name="psA", bufs=2, space="PSUM"))

        # ---- phase 1: LN1 stats only (full width) -> tiny stats AG ----
        with nc.named_scope("ln1"):
            sx = psum.tile([1, TOK], f32, tag="sm1", bufs=1, name="l1sx")
            for k in range(KD):
                nc.tensor.matmul(sx, ones_cb, xT_sb[:, k, :],
                                 start=(k == 0), stop=(k == KD - 1))
            sxx = psum.tile([1, TOK], f32, tag="sm2", bufs=1, name="l1sxx")
            for k in range(KD):
                sq = sqp.tile([P, TOK], bf16, tag="sq")
                nc.vector.tensor_mul(sq, xT_sb[:, k, :], xT_sb[:, k, :])
                nc.tensor.matmul(sxx, ones_cb, sq,
                                 start=(k == 0), stop=(k == KD - 1))
            muf = rows.tile([1, TOK], f32, tag="row")
            nc.vector.tensor_scalar_mul(muf, sx, 1.0 / D)
            m2 = rows.tile([1, TOK], f32, tag="row")
            nc.vector.tensor_scalar_mul(m2, sxx, 1.0 / D)
            var = rows.tile([1, TOK], f32, tag="row")
            nc.vector.tensor_tensor(out=var, in0=muf, in1=muf, op=OP.mult)
            nc.vector.tensor_tensor(out=var, in0=m2, in1=var, op=OP.subtract)
            lnv = rows.tile([1, TOK], f32, tag="row")
            nc.scalar.activation(lnv, var, AF.Ln, bias=eps_sb[:])
            rstd = rows.tile([1, TOK], bf16, tag="rowb")
            with nc.allow_low_precision(reason="bf16 rstd broadcast"):
                nc.scalar.activation(rstd, lnv, AF.Exp, scale=-0.5)
            mub16 = rows.tile([1, TOK], bf16, tag="rowb")
            with nc.allow_low_precision(reason="bf16 mu broadcast"):
                nc.vector.tensor_copy(mub16, muf)
            mub = psum.tile([P, TOK], f32, tag="big", bufs=2, name="l1mub")
            nc.tensor.matmul(mub, ones_rb, mub16, start=True, stop=True)
            rsb = psum.tile([P, TOK], f32, tag="big", bufs=2, name="l1rsb")
            nc.tensor.matmul(rsb, ones_rb, rstd, start=True, stop=True)
            xn = res.tile([P, KD, TOK], bf16)
            for k in range(KD):
                t1 = lnt.tile([P, TOK], bf16, tag="ln1")
                nc.vector.tensor_tensor(out=t1, in0=xT_sb[:, k, :], in1=mub,
                                        op=OP.subtract)
                nc.vector.tensor_tensor(out=xn[:, k, :], in0=t1, in1=rsb,
                                        op=OP.mult)
                nc.sync.dma_start(cc1_in[:, k, :], xn[:, k, :])
        with nc.named_scope("agx"):
            nc.gpsimd.collective_compute(
                "AllGather", OP.bypass, replica_groups=RG,
                ins=[cc1_in[:].opt()], outs=[cc1_out[:].opt()])

        # ---- phase 2+3: QKV (per rank) interleaved with attention ----
        kT = [ares.tile([P, TOK], bf16, name=f"kT{r}") for r in range(NCORES)]
        vA = [ares.tile([P, 4, HL, DH + 1], bf16, name=f"vA{r}")
              for r in range(NCORES)]
        qT = [ares.tile([P, TOK], bf16, name=f"qT{r}") for r in range(NCORES)]
        for r in range(NCORES):
            nc.vector.memset(vA[r][:, :, :, DH:DH + 1], 1.0)
        aT = ares.tile([P, NCORES, TOK], bf16)
        dT = ares.tile([1, HL, NCORES, TOK], bf16)

        def qkv_unit(r):
            with nc.named_scope(f"qkv{r}"):
                xn_r = xnp.tile([P, KD, TOK], bf16, tag="xnr")
                for kh in range(2):
                    nc.sync.dma_start(xn_r[:, 4 * kh:4 * (kh + 1), :],
                                      cc1_out[r][:, 4 * kh:4 * (kh + 1), :])
                for which in range(2):  # 0 -> q, 1 -> k
                    ps = psum.tile([P, TOK], f32, tag="big", bufs=2)
                    cb = which * P
                    for k in range(KD):
                        nc.tensor.matmul(ps, aw_sb[:, k, cb:cb + P],
                                         xn_r[:, k, :],
                                         start=(k == 0), stop=(k == KD - 1))
                    dst = qT[r] if which == 0 else kT[r]
                    bias = qbias if which == 0 else kbias
                    nc.scalar.activation(dst, ps, AF.Identity, bias=bias[:])
                for t in range(4):
                    psv = psum.tile([P, P], f32, tag="sm1", bufs=1)
                    for k in range(KD):
                        nc.tensor.matmul(psv, xn_r[:, k, P * t:P * (t + 1)],
                                         aw_sb[:, k, 2 * P:3 * P],
                                         start=(k == 0), stop=(k == KD - 1))
                    nc.vector.tensor_tensor(
                        out=vA[r][:, t, :, 0:DH],
                        in0=psv.rearrange("p (h d) -> p h d", h=HL),
                        in1=vb_bc.rearrange("p (h d) -> p h d", h=HL),
                        op=OP.add)

        def attn_unit(b, qc):
            """Causal attention chunk: UNnormalized numerators into aT,
            reciprocal softmax denominators into dT (divide post-A2A).
            Software-pipelined: scores(pair p+1) overlap exp(pair p)."""
            qr = 4 * b + qc // 2
            qo = QCH * (qc % 2)
            nkb = 2 * qc + 2
            npair = nkb // 2
            accs = [psA.tile([DH + 1, QCH], f32, tag="acc", bufs=2,
                             name=f"acc{b}_{qc}_{h}")
                    for h in range(HL)]

            def emit_scores(p):
                kb0 = 2 * p
                ws = []
                for h in range(HL):
                    hb = DH * h
                    sc = psA.tile([P, 2 * QCH], f32, tag="sc", bufs=2)
                    for j in range(2):
                        kb = kb0 + j
                        r = 4 * b + kb // 4
                        t = kb % 4
                        nc.tensor.matmul(
                            sc[:, QCH * j:QCH * (j + 1)],
                            kT[r][hb:hb + DH, P * t:P * (t + 1)],
                            qT[qr][hb:hb + DH, qo:qo + QCH],
                            start=True, stop=True,
                            skip_group_check=True)
                    w = wp.tile([P, 2 * QCH], bf16, tag="w")
                    nc.scalar.activation(w, sc, AF.Exp, scale=0.125)
                    if kb0 == 2 * qc:  # diagonal pair: apply masks
                        nc.vector.tensor_mul(
                            w.rearrange("p (m q) -> p m q", m=2),
                            w.rearrange("p (m q) -> p m q", m=2),
                            mkc)
                    ws.append(w)
                return ws

            def emit_av(p, ws):
                kb0 = 2 * p
                for h in range(HL):
                    for j in range(2):
                        kb = kb0 + j
                        r = 4 * b + kb // 4
                        t = kb % 4
                        nc.tensor.matmul(
                            accs[h], vA[r][:, t, h, :],
                            ws[h][:, QCH * j:QCH * (j + 1)],
                            start=(kb == 0), stop=(kb == nkb - 1),
                            skip_group_check=True)

            ws_p = emit_scores(0)
            for p in range(npair):
                ws_n = emit_scores(p + 1) if p + 1 < npair else None
                emit_av(p, ws_p)
                ws_p = ws_n
            for h in range(HL):
                hb = DH * h
                nc.vector.tensor_copy(aT[hb:hb + DH, qr, qo:qo + QCH],
                                      accs[h][0:DH, :])
                nc.vector.tensor_copy(dT[0:1, h, qr, qo:qo + QCH],
                                      accs[h][DH:DH + 1, :])
            if qc % 2 == 1:  # chunk qr complete: stage its A2A slot early
                nc.sync.dma_start(cc2_in[qr, 0:P], aT[:, qr, :])
                nc.sync.dma_start(cc2_in[qr, P:P + 2], dT[0:1, :, qr, :])

        with nc.named_scope("qkv_attn"):
            qkv_unit(0)
            for b in range(B):
                for qc in range(NQC):
                    attn_unit(b, qc)
                    if qc % 2 == 0:
                        r = 4 * b + qc // 2 + 1
                        if r < NCORES:
                            qkv_unit(r)

        # ---- phase 4: AllToAll back to token-parallel ----
        with nc.named_scope("a2a"):
            nc.gpsimd.collective_compute(
                "AllToAll", OP.bypass, replica_groups=RG,
                ins=[cc2_in[:].opt()], outs=[cc2_out[:].opt()])
        # latency-critical result loads: emit BEFORE the MLP weight
        # avalanche (s1.close frees SBUF and unleashes wgt prefetch DMAs)
        aF = [dfp.tile([P, TOK], bf16, tag="aF", name=f"aF{r}")
              for r in range(NCORES)]
        aFn = [dfp.tile([P, TOK], bf16, tag="aFn", name=f"aFn{r}")
               for r in range(NCORES)]
        dfh = [dfp.tile([1, 2 * TOK], bf16, tag="dfh", name=f"dfh{r}")
               for r in range(NCORES)]
        with nc.named_scope("resload"):
            for r in range(NCORES):
                nc.sync.dma_start(aF[r], cc2_out[r, 0:P])
                nc.sync.dma_start(dfh[r][:, 0:TOK], cc2_out[r, P:P + 1])
                nc.sync.dma_start(dfh[r][:, TOK:2 * TOK],
                                  cc2_out[r, P + 1:P + 2])
            rcps = []
            for r in range(NCORES):
                rcp = dfp.tile([1, 2 * TOK], bf16, tag="rcp", name=f"rcp{r}")
                with nc.allow_low_precision(reason="bf16 softmax denom"):
                    nc.vector.reciprocal(rcp, dfh[r])
                rcps.append(rcp)

        s1.close()  # release attention-era SBUF
        psB = ctx.enter_context(tc.tile_pool(name="psB", bufs=4, space="PSUM"))
        mlp = ctx.enter_context(tc.tile_pool(name="mlp", bufs=1))
        wgt = ctx.enter_context(tc.tile_pool(name="wgt", bufs=1))
        outp = ctx.enter_context(tc.tile_pool(name="outp", bufs=2))

        # ---- phase 5: softmax normalize + output projection + residual ----
        h1T = mlp.tile([P, KD, TOK], bf16)
        with nc.named_scope("proj"):
            for r in range(NCORES):
                rcp = rcps[r]
                rb0 = psB.tile([DH, TOK], f32, tag="rb",
                               bufs=4, name=f"rb0_{r}")
                nc.tensor.matmul(rb0, ones_rb[:, 0:DH], rcp[:, 0:TOK],
                                 start=True, stop=True, skip_group_check=True)
                rb1 = psB.tile([DH, TOK], f32, tag="rb",
                               bufs=4, name=f"rb1_{r}")
                nc.tensor.matmul(rb1, ones_rb[:, 0:DH], rcp[:, TOK:],
                                 start=True, stop=True, skip_group_check=True)
                nc.vector.tensor_mul(aFn[r][0:DH, :], aF[r][0:DH, :], rb0)
                nc.vector.tensor_mul(aFn[r][DH:P, :], aF[r][DH:P, :], rb1)
            # proj matmuls + incremental LN2 stats (chained across f)
            sx2 = psum.tile([1, TOK], f32, tag="sm1", bufs=1, name="l2sx")
            sxx2 = psum.tile([1, TOK], f32, tag="sm2", bufs=1, name="l2sxx")
            for f in range(KD):
                pwt = wgt.tile([P, KD, P], bf16, tag="pw", bufs=2)
                nc.sync.dma_start(pwt, pw[f])
                ps = psum.tile([P, TOK], f32, tag="big", bufs=2)
                for k in range(KD):
                    nc.tensor.matmul(ps, pwt[:, k, :], aFn[k],
                                     start=(k == 0), stop=(k == KD - 1))
                t1 = lnt.tile([P, TOK], f32, tag="pj")
                nc.vector.tensor_scalar_add(t1, ps, pb_sb[:, f:f + 1])
                nc.vector.tensor_tensor(out=h1T[:, f, :], in0=t1,
                                        in1=xT_sb[:, f, :], op=OP.add)
                nc.tensor.matmul(sx2, ones_cb, h1T[:, f, :],
                                 start=(f == 0), stop=(f == KD - 1))
                sq2 = sqp.tile([P, TOK], bf16, tag="sq2")
                nc.vector.tensor_mul(sq2, h1T[:, f, :], h1T[:, f, :])
                nc.tensor.matmul(sxx2, ones_cb, sq2,
                                 start=(f == 0), stop=(f == KD - 1))

        # ---- phase 6: LN2 finish (stats already accumulated) ----
        mT = mlp.tile([P, KD, TOK], bf16)
        with nc.named_scope("ln2"):
            muf = rows.tile([1, TOK], f32, tag="row")
            nc.vector.tensor_scalar_mul(muf, sx2, 1.0 / D)
            m2 = rows.tile([1, TOK], f32, tag="row")
            nc.vector.tensor_scalar_mul(m2, sxx2, 1.0 / D)
            var = rows.tile([1, TOK], f32, tag="row")
            nc.vector.tensor_tensor(out=var, in0=muf, in1=muf, op=OP.mult)
            nc.vector.tensor_tensor(out=var, in0=m2, in1=var, op=OP.subtract)
            lnv = rows.tile([1, TOK], f32, tag="row")
            nc.scalar.activation(lnv, var, AF.Ln, bias=eps_sb[:])
            rstd = rows.tile([1, TOK], bf16, tag="rowb")
            with nc.allow_low_precision(reason="bf16 rstd broadcast"):
                nc.scalar.activation(rstd, lnv, AF.Exp, scale=-0.5)
            mub16 = rows.tile([1, TOK], bf16, tag="rowb")
            with nc.allow_low_precision(reason="bf16 mu broadcast"):
                nc.vector.tensor_copy(mub16, muf)
            mub = psum.tile([P, TOK], f32, tag="big", bufs=2, name="l2mub")
            nc.tensor.matmul(mub, ones_rb, mub16, start=True, stop=True)
            rsb = psum.tile([P, TOK], f32, tag="big", bufs=2, name="l2rsb")
            nc.tensor.matmul(rsb, ones_rb, rstd, start=True, stop=True)
            for k in range(KD):
                t1 = lnt.tile([P, TOK], bf16, tag="ln2")
                nc.vector.tensor_tensor(out=t1, in0=h1T[:, k, :], in1=mub,
                                        op=OP.subtract)
                nc.vector.tensor_tensor(out=mT[:, k, :], in0=t1, in1=rsb,
                                        op=OP.mult)

        # ---- phase 7: MLP ----
        hT = mlp.tile([P, KDI, TOK], bf16)
        with nc.named_scope("fc1"):
            for j in range(KDI):
                fwt = wgt.tile([P, KD, P], bf16, tag="fw", bufs=3)
                nc.sync.dma_start(fwt, fw[j])
                ps = psum.tile([P, TOK], f32, tag="big", bufs=2)
                for k in range(KD):
                    nc.tensor.matmul(ps, fwt[:, k, :], mT[:, k, :],
                                     start=(k == 0), stop=(k == KD - 1))
                nc.scalar.activation(hT[:, j, :], ps, AF.Gelu_apprx_tanh,
                                     bias=fb_sb[:, j:j + 1])
        with nc.named_scope("fc2"):
            for f in range(KD):
                gwt = wgt.tile([P, KDI, P], bf16, tag="gw", bufs=2)
                nc.sync.dma_start(gwt, gw[f])
                ps = psum.tile([P, TOK], f32, tag="big", bufs=2)
                for k in range(KDI):
                    nc.tensor.matmul(ps, gwt[:, k, :], hT[:, k, :],
                                     start=(k == 0), stop=(k == KDI - 1))
                o = outp.tile([P, TOK], f32, tag="ot")
                nc.vector.tensor_scalar_add(o, ps, gb_sb[:, f:f + 1])
                nc.vector.tensor_tensor(out=o, in0=o, in1=h1T[:, f, :],
                                        op=OP.add)
                nc.sync.dma_start(outT[P * f:P * (f + 1), :], o)

    nc.compile()
    return nc


def shard_inputs(inputs):
    """Full inputs -> list of 8 per-core input dicts (host-side layout only)."""
    bf16 = ml_dtypes.bfloat16
    f32 = np.float32
    hs = np.asarray(inputs["hidden_states"], f32).reshape(TT, D)
    l1g = np.asarray(inputs["ln1_g"], f32)
    l1b = np.asarray(inputs["ln1_b"], f32)
    l2g = np.asarray(inputs["ln2_g"], f32)
    l2b = np.asarray(inputs["ln2_b"], f32)
    # fold LN1 gamma/beta into attn_w/attn_b, LN2 into fc_w/fc_b
    attn_w = np.asarray(inputs["attn_w"], f32) * l1g[:, None]
    attn_b = np.asarray(inputs["attn_b"], f32) + l1b @ np.asarray(inputs["attn_w"], f32)
    fc_w = np.asarray(inputs["fc_w"], f32) * l2g[:, None]
    fc_b = np.asarray(inputs["fc_b"], f32) + l2b @ np.asarray(inputs["fc_w"], f32)

    def col(v):  # [D] -> [P, KD]
        return np.ascontiguousarray(np.asarray(v, f32).reshape(KD, P).T)

    pw = np.ascontiguousarray(np.asarray(inputs["proj_w"], f32)
                              .reshape(KD, P, KD, P).transpose(2, 1, 0, 3)
                              .astype(bf16))
    fw = np.ascontiguousarray(fc_w.reshape(KD, P, KDI, P).transpose(2, 1, 0, 3)
                              .astype(bf16))
    gw = np.ascontiguousarray(np.asarray(inputs["fc2_w"], f32)
                              .reshape(KDI, P, KD, P).transpose(2, 1, 0, 3)
                              .astype(bf16))
    pb = col(inputs["proj_b"])
    fbv = np.ascontiguousarray(fc_b.reshape(KDI, P).T)
    gbv = col(inputs["fc2_b"])

    ii, jj = np.meshgrid(np.arange(P), np.arange(QCH), indexing="ij")
    mkv = np.stack([(jj >= ii), (jj >= ii + P)]).astype(bf16)

    maps = []
    for c in range(NCORES):
        cols = np.r_[P * c:P * (c + 1),
                     D + P * c:D + P * (c + 1),
                     2 * D + P * c:2 * D + P * (c + 1)]
        aw_c = np.ascontiguousarray(
            attn_w[:, cols].reshape(KD, P, 3 * P).transpose(1, 0, 2)
            .astype(bf16))
        ab_c = np.ascontiguousarray(attn_b[cols], dtype=f32)
        xT_c = np.ascontiguousarray(
            hs[TOK * c:TOK * (c + 1)].T.reshape(KD, P, TOK).transpose(1, 0, 2)
        ).astype(bf16)
        maps.append({
            "xT": xT_c, "aw": aw_c, "ab": ab_c,
            "pw": pw, "pb": pb, "fw": fw, "fb": fbv, "gw": gw, "gb": gbv,
            "mk": mkv,
        })
    return maps


def unshard(results):
    out = np.concatenate([np.asarray(r["outT"]).T for r in results], axis=0)
    return np.ascontiguousarray(out.reshape(B, S, D))


def kernel(**inputs):
    global _CACHED_NC
    from concourse.bass_utils import run_bass_kernel_spmd
    if _CACHED_NC is None:
        _CACHED_NC = build_nc()
    in_maps = shard_inputs(inputs)
    res = run_bass_kernel_spmd(_CACHED_NC, in_maps,
                               core_ids=list(range(NCORES)))
    return unshard(res.results)


# revision 24
# speedup vs baseline: 1.0249x; 1.0082x over previous
"""Trainium2 Bass kernel for a GPT-2 style transformer block (nn_Block_16690242913196).

Sharding (8 NeuronCores, identical SPMD program):
  - LN1/QKV/proj/LN2/MLP: token-parallel (core i owns 512 flat tokens).
  - Attention: head-parallel (core i owns heads {2i, 2i+1}, all tokens).
  - Collective 1: AllGather of the RAW bf16 input (1MB/rank), triggered
    at t~0 straight from DRAM — no LN dependency, so it absorbs the
    cross-core start skew under useful work.
  - Collective 2: tiny AllGather of LN1 stats rows (mu, rstd; 2KB).
    Each consumer normalizes rank r's activations itself (2 broadcast
    matmuls + 16 bf16 DVE ops per rank, hidden in the QKV/attention era).
  - Collective 3: AllToAll of UNnormalized attention outputs plus
    RECIPROCAL softmax denominators (130 rows x 512 bf16); the divide
    becomes a broadcast-matmul + multiply on the token-parallel side.

  LN gamma/beta are folded into the consuming weights host-side. rstd
  is computed as exp(-0.5*ln(var+eps)) so LNs and the attention exp
  share one ACT table set. LN2 stats accumulate inside the proj loop.
"""

import numpy as np
import ml_dtypes

P = 128
B, S, D, H = 2, 2048, 1024, 16
DH = D // H          # 64
DI = 4 * D           # 4096
EPS = 1e-5
NCORES = 8
TT = B * S           # 4096 flat tokens
TOK = TT // NCORES   # 512 tokens per core
KD = D // P          # 8
KDI = DI // P        # 32
QCH = 256            # query chunk (2 blocks of 128)
NQC = S // QCH       # 8 query chunks per batch
HL = H // NCORES     # 2 local heads
RG = [list(range(NCORES))]

_CACHED_NC = None


def build_nc():
    import concourse.bacc as bacc
    import concourse.tile as tile
    import concourse.mybir as mybir
    from contextlib import ExitStack

    dt = mybir.dt
    f32, bf16, f32r = dt.float32, dt.bfloat16, dt.float32r
    f8 = dt.float8e4
    DR = mybir.MatmulPerfMode.DoubleRow
    AF = mybir.ActivationFunctionType
    OP = mybir.AluOpType

    nc = bacc.Bacc("TRN2", target_bir_lowering=False, debug=False,
                   num_devices=NCORES)

    # ---- kernel I/O (per-core shapes) ----
    xT = nc.dram_tensor("xT", [P, KD, TOK], bf16, kind="ExternalInput").ap()
    aw = nc.dram_tensor("aw", [P, KD, 3 * P], bf16, kind="ExternalInput").ap()
    ab = nc.dram_tensor("ab", [3 * P], f32, kind="ExternalInput").ap()
    pw = nc.dram_tensor("pw", [KD, P, KD, P], bf16, kind="ExternalInput").ap()
    pb = nc.dram_tensor("pb", [P, KD], f32, kind="ExternalInput").ap()
    fw = nc.dram_tensor("fw", [KDI, P, KD, P], bf16, kind="ExternalInput").ap()
    fb = nc.dram_tensor("fb", [P, KDI], f32, kind="ExternalInput").ap()
    gw = nc.dram_tensor("gw", [KD, P, KDI, P], bf16, kind="ExternalInput").ap()
    gb = nc.dram_tensor("gb", [P, KD], f32, kind="ExternalInput").ap()
    mk = nc.dram_tensor("mk", [2, P, QCH], bf16, kind="ExternalInput").ap()
    outT = nc.dram_tensor("outT", [D, TOK], f32, kind="ExternalOutput").ap()

    with tile.TileContext(nc) as tc, ExitStack() as ctx:
        const = ctx.enter_context(tc.tile_pool(name="const", bufs=1))
        dram = ctx.enter_context(tc.tile_pool(name="dram", bufs=1, space="DRAM"))
        psum = ctx.enter_context(tc.tile_pool(name="psum", bufs=1, space="PSUM"))
        rows = ctx.enter_context(tc.tile_pool(name="rows", bufs=6))
        sqp = ctx.enter_context(tc.tile_pool(name="sqp", bufs=2))
        lnt = ctx.enter_context(tc.tile_pool(name="lnt", bufs=3))
        dfp = ctx.enter_context(tc.tile_pool(name="dfp", bufs=8))
        res = ctx.enter_context(tc.tile_pool(name="res", bufs=1))

        # collective bounce buffers
        cc1_in = dram.tile([P, KD, TOK], bf16)
        cc1_out = dram.tile([NCORES, P, KD, TOK], bf16, addr_space="Shared")
        cc2_in = dram.tile([NCORES, P + 2, TOK], bf16)
        cc2_out = dram.tile([NCORES, P + 2, TOK], bf16)

        # input activations into SBUF (split per-k across DMA queues)
        xT_sb = res.tile([P, KD, TOK], bf16)
        for k in range(KD):
            nc.sync.dma_start(xT_sb[:, k, :], xT[:, k, :])

        # ---- constants in SBUF ----
        aw_sb = const.tile([P, KD, 3 * P], bf16)
        nc.sync.dma_start(aw_sb, aw)
        qbias = const.tile([P, 1], f32)
        nc.sync.dma_start(qbias, ab[0:P].rearrange("(p o) -> p o", o=1))
        kbias = const.tile([P, 1], f32)
        nc.sync.dma_start(kbias, ab[P:2 * P].rearrange("(p o) -> p o", o=1))
        vb_bc = const.tile([P, P], f32)
        nc.sync.dma_start(vb_bc, ab[2 * P:3 * P].rearrange("(o c) -> o c", o=1).to_broadcast((P, P)))
        mkc = const.tile([P, 2, QCH], bf16)
        nc.sync.dma_start(mkc, mk.rearrange("m p q -> p m q"))
        pb_sb = const.tile([P, KD], f32)
        nc.sync.dma_start(pb_sb, pb)
        fb_sb = const.tile([P, KDI], f32)
        nc.sync.dma_start(fb_sb, fb)
        gb_sb = const.tile([P, KD], f32)
        nc.sync.dma_start(gb_sb, gb)
        ones_cf = const.tile([P, 1], f32)
        nc.vector.memset(ones_cf, 1.0)
        ones_c = const.tile([P, 1], f32r)
        nc.vector.tensor_copy(ones_c, ones_cf)
        ones_cb = const.tile([P, 1], bf16)
        nc.vector.tensor_copy(ones_cb, ones_cf)
        ones_rf = const.tile([1, P], f32)
        nc.vector.memset(ones_rf, 1.0)
        ones_r = const.tile([1, P], f32r)
        nc.vector.tensor_copy(ones_r, ones_rf)
        ones_rb = const.tile([1, P], bf16)
        nc.vector.tensor_copy(ones_rb, ones_rf)
        eps_sb = const.tile([1, 1], f32)
        nc.vector.memset(eps_sb, EPS)

        # phase-scoped pools: attention-era tiles are freed before the MLP
        # pools (notably the 4MB hT) allocate.
        s1 = ExitStack()
        ares = s1.enter_context(tc.tile_pool(name="ares", bufs=1))
        xnp = s1.enter_context(tc.tile_pool(name="xnp", bufs=2))
        wp = s1.enter_context(tc.tile_pool(name="wp", bufs=4))
        psA = s1.enter_context(tc.tile_pool(name="psA", bufs=2, space="PSUM"))

        # ---- phase 1: LN1 stats only (full width) -> tiny stats AG ----
        with nc.named_scope("ln1"):
            sx = psum.tile([1, TOK], f32, tag="sm1", bufs=1, name="l1sx")
            for k in range(KD):
                nc.tensor.matmul(sx, ones_cb, xT_sb[:, k, :],
                                 start=(k == 0), stop=(k == KD - 1))
            sxx = psum.tile([1, TOK], f32, tag="sm2", bufs=1, name="l1sxx")
            for k in range(KD):
                sq = sqp.tile([P, TOK], bf16, tag="sq")
                nc.vector.tensor_mul(sq, xT_sb[:, k, :], xT_sb[:, k, :])
                nc.tensor.matmul(sxx, ones_cb, sq,
                                 start=(k == 0), stop=(k == KD - 1))
            muf = rows.tile([1, TOK], f32, tag="row")
            nc.vector.tensor_scalar_mul(muf, sx, 1.0 / D)
            m2 = rows.tile([1, TOK], f32, tag="row")
            nc.vector.tensor_scalar_mul(m2, sxx, 1.0 / D)
            var = rows.tile([1, TOK], f32, tag="row")
            nc.vector.tensor_tensor(out=var, in0=muf, in1=muf, op=OP.mult)
            nc.vector.tensor_tensor(out=var, in0=m2, in1=var, op=OP.subtract)
            lnv = rows.tile([1, TOK], f32, tag="row")
            nc.scalar.activation(lnv, var, AF.Ln, bias=eps_sb[:])
            rstd = rows.tile([1, TOK], bf16, tag="rowb")
            with nc.allow_low_precision(reason="bf16 rstd broadcast"):
                nc.scalar.activation(rstd, lnv, AF.Exp, scale=-0.5)
            mub16 = rows.tile([1, TOK], bf16, tag="rowb")
            with nc.allow_low_precision(reason="bf16 mu broadcast"):
                nc.vector.tensor_copy(mub16, muf)
            mub = psum.tile([P, TOK], f32, tag="big", bufs=2, name="l1mub")
            nc.tensor.matmul(mub, ones_rb, mub16, start=True, stop=True)
            rsb = psum.tile([P, TOK], f32, tag="big", bufs=2, name="l1rsb")
            nc.tensor.matmul(rsb, ones_rb, rstd, start=True, stop=True)
            xn = res.tile([P, KD, TOK], bf16)
            for k in range(KD):
                t1 = lnt.tile([P, TOK], bf16, tag="ln1")
                nc.vector.tensor_tensor(out=t1, in0=xT_sb[:, k, :], in1=mub,
                                        op=OP.subtract)
                nc.vector.tensor_tensor(out=xn[:, k, :], in0=t1, in1=rsb,
                                        op=OP.mult)
                nc.sync.dma_start(cc1_in[:, k, :], xn[:, k, :])
        with nc.named_scope("agx"):
            nc.gpsimd.collective_compute(
                "AllGather", OP.bypass, replica_groups=RG,
                ins=[cc1_in[:].opt()], outs=[cc1_out[:].opt()])

        # ---- phase 2+3: QKV (per rank) interleaved with attention ----
        kT = [ares.tile([P, TOK], bf16, name=f"kT{r}") for r in range(NCORES)]
        vA = [ares.tile([P, 4, HL, DH + 1], bf16, name=f"vA{r}")
              for r in range(NCORES)]
        qT = [ares.tile([P, TOK], bf16, name=f"qT{r}") for r in range(NCORES)]
        for r in range(NCORES):
            nc.vector.memset(vA[r][:, :, :, DH:DH + 1], 1.0)
        aT = ares.tile([P, NCORES, TOK], bf16)
        dT = ares.tile([1, HL, NCORES, TOK], bf16)
        dTr = ares.tile([1, HL, NCORES, TOK], bf16)

        def qkv_unit(r):
            with nc.named_scope(f"qkv{r}"):
                xn_r = xnp.tile([P, KD, TOK], bf16, tag="xnr")
                for kh in range(2):
                    nc.sync.dma_start(xn_r[:, 4 * kh:4 * (kh + 1), :],
                                      cc1_out[r][:, 4 * kh:4 * (kh + 1), :])
                for which in range(2):  # 0 -> q, 1 -> k
                    ps = psum.tile([P, TOK], f32, tag="big", bufs=2)
                    cb = which * P
                    for k in range(KD):
                        nc.tensor.matmul(ps, aw_sb[:, k, cb:cb + P],
                                         xn_r[:, k, :],
                                         start=(k == 0), stop=(k == KD - 1))
                    dst = qT[r] if which == 0 else kT[r]
                    bias = qbias if which == 0 else kbias
                    nc.scalar.activation(dst, ps, AF.Identity, bias=bias[:])
                for t in range(4):
                    psv = psum.tile([P, P], f32, tag="sm1", bufs=1)
                    for k in range(KD):
                        nc.tensor.matmul(psv, xn_r[:, k, P * t:P * (t + 1)],
                                         aw_sb[:, k, 2 * P:3 * P],
                                         start=(k == 0), stop=(k == KD - 1))
                    nc.vector.tensor_tensor(
                        out=vA[r][:, t, :, 0:DH],
                        in0=psv.rearrange("p (h d) -> p h d", h=HL),
                        in1=vb_bc.rearrange("p (h d) -> p h d", h=HL),
                        op=OP.add)

        def attn_unit(b, qc):
            """Causal attention chunk: UNnormalized numerators into aT,
            reciprocal softmax denominators into dT (divide post-A2A).
            Software-pipelined: scores(pair p+1) overlap exp(pair p)."""
            qr = 4 * b + qc // 2
            qo = QCH * (qc % 2)
            nkb = 2 * qc + 2
            npair = nkb // 2
            accs = [psA.tile([DH + 1, QCH], f32, tag="acc", bufs=2,
                             name=f"acc{b}_{qc}_{h}")
                    for h in range(HL)]

            def emit_scores(p):
                kb0 = 2 * p
                ws = []
                for h in range(HL):
                    hb = DH * h
                    sc = psA.tile([P, 2 * QCH], f32, tag="sc", bufs=2)
                    for j in range(2):
                        kb = kb0 + j
                        r = 4 * b + kb // 4
                        t = kb % 4
                        nc.tensor.matmul(
                            sc[:, QCH * j:QCH * (j + 1)],
                            kT[r][hb:hb + DH, P * t:P * (t + 1)],
                            qT[qr][hb:hb + DH, qo:qo + QCH],
                            start=True, stop=True,
                            skip_group_check=True)
                    w = wp.tile([P, 2 * QCH], bf16, tag="w")
                    nc.scalar.activation(w, sc, AF.Exp, scale=0.125)
                    if kb0 == 2 * qc:  # diagonal pair: apply masks
                        nc.vector.tensor_mul(
                            w.rearrange("p (m q) -> p m q", m=2),
                            w.rearrange("p (m q) -> p m q", m=2),
                            mkc)
                    ws.append(w)
                return ws

            def emit_av(p, ws):
                kb0 = 2 * p
                for h in range(HL):
                    for j in range(2):
                        kb = kb0 + j
                        r = 4 * b + kb // 4
                        t = kb % 4
                        nc.tensor.matmul(
                            accs[h], vA[r][:, t, h, :],
                            ws[h][:, QCH * j:QCH * (j + 1)],
                            start=(kb == 0), stop=(kb == nkb - 1),
                            skip_group_check=True)

            ws_p = emit_scores(0)
            for p in range(npair):
                ws_n = emit_scores(p + 1) if p + 1 < npair else None
                emit_av(p, ws_p)
                ws_p = ws_n
            for h in range(HL):
                hb = DH * h
                nc.vector.tensor_copy(aT[hb:hb + DH, qr, qo:qo + QCH],
                                      accs[h][0:DH, :])
                nc.vector.tensor_copy(dT[0:1, h, qr, qo:qo + QCH],
                                      accs[h][DH:DH + 1, :])
            if qc % 2 == 1:  # chunk qr complete: recip + stage its slot
                for h in range(HL):
                    with nc.allow_low_precision(reason="bf16 softmax denom"):
                        nc.vector.reciprocal(dTr[0:1, h, qr, :],
                                             dT[0:1, h, qr, :])
                nc.sync.dma_start(cc2_in[qr, 0:P], aT[:, qr, :])
                nc.sync.dma_start(cc2_in[qr, P:P + 2], dTr[0:1, :, qr, :])

        with nc.named_scope("qkv_attn"):
            qkv_unit(0)
            for b in range(B):
                for qc in range(NQC):
                    attn_unit(b, qc)
                    if qc % 2 == 0:
                        r = 4 * b + qc // 2 + 1
                        if r < NCORES:
                            qkv_unit(r)

        # ---- phase 4: AllToAll back to token-parallel ----
        with nc.named_scope("a2a"):
            nc.gpsimd.collective_compute(
                "AllToAll", OP.bypass, replica_groups=RG,
                ins=[cc2_in[:].opt()], outs=[cc2_out[:].opt()])
        # latency-critical result loads: emit BEFORE the MLP weight
        # avalanche (s1.close frees SBUF and unleashes wgt prefetch DMAs)
        aF = [dfp.tile([P, TOK], bf16, tag="aF", name=f"aF{r}")
              for r in range(NCORES)]
        aFn = [dfp.tile([P, TOK], bf16, tag="aFn", name=f"aFn{r}")
               for r in range(NCORES)]
        dfh = [dfp.tile([1, 2 * TOK], bf16, tag="dfh", name=f"dfh{r}")
               for r in range(NCORES)]
        with nc.named_scope("resload"):
            for r in range(NCORES):
                nc.sync.dma_start(aF[r], cc2_out[r, 0:P])
                nc.sync.dma_start(dfh[r][:, 0:TOK], cc2_out[r, P:P + 1])
                nc.sync.dma_start(dfh[r][:, TOK:2 * TOK],
                                  cc2_out[r, P + 1:P + 2])

        s1.close()  # release attention-era SBUF
        psB = ctx.enter_context(tc.tile_pool(name="psB", bufs=4, space="PSUM"))
        mlp = ctx.enter_context(tc.tile_pool(name="mlp", bufs=1))
        wgt = ctx.enter_context(tc.tile_pool(name="wgt", bufs=1))
        outp = ctx.enter_context(tc.tile_pool(name="outp", bufs=2))

        # ---- phase 5: softmax normalize + output projection + residual ----
        h1T = mlp.tile([P, KD, TOK], bf16)
        with nc.named_scope("proj"):
            for r in range(NCORES):
                rb0 = psB.tile([DH, TOK], f32, tag="rb",
                               bufs=4, name=f"rb0_{r}")
                nc.tensor.matmul(rb0, ones_rb[:, 0:DH], dfh[r][:, 0:TOK],
                                 start=True, stop=True, skip_group_check=True)
                rb1 = psB.tile([DH, TOK], f32, tag="rb",
                               bufs=4, name=f"rb1_{r}")
                nc.tensor.matmul(rb1, ones_rb[:, 0:DH], dfh[r][:, TOK:],
                                 start=True, stop=True, skip_group_check=True)
                nc.vector.tensor_mul(aFn[r][0:DH, :], aF[r][0:DH, :], rb0)
                nc.vector.tensor_mul(aFn[r][DH:P, :], aF[r][DH:P, :], rb1)
            # proj matmuls + incremental LN2 stats (chained across f)
            sx2 = psum.tile([1, TOK], f32, tag="sm1", bufs=1, name="l2sx")
            sxx2 = psum.tile([1, TOK], f32, tag="sm2", bufs=1, name="l2sxx")
            for f in range(KD):
                pwt = wgt.tile([P, KD, P], bf16, tag="pw", bufs=2)
                nc.sync.dma_start(pwt, pw[f])
                ps = psum.tile([P, TOK], f32, tag="big", bufs=2)
                for k in range(KD):
                    nc.tensor.matmul(ps, pwt[:, k, :], aFn[k],
                                     start=(k == 0), stop=(k == KD - 1))
                t1 = lnt.tile([P, TOK], f32, tag="pj")
                nc.vector.tensor_scalar_add(t1, ps, pb_sb[:, f:f + 1])
                nc.vector.tensor_tensor(out=h1T[:, f, :], in0=t1,
                                        in1=xT_sb[:, f, :], op=OP.add)
                nc.tensor.matmul(sx2, ones_cb, h1T[:, f, :],
                                 start=(f == 0), stop=(f == KD - 1))
                sq2 = sqp.tile([P, TOK], bf16, tag="sq2")
                nc.vector.tensor_mul(sq2, h1T[:, f, :], h1T[:, f, :])
                nc.tensor.matmul(sxx2, ones_cb, sq2,
                                 start=(f == 0), stop=(f == KD - 1))

        # ---- phase 6: LN2 finish (stats already accumulated) ----
        mT = mlp.tile([P, KD, TOK], bf16)
        with nc.named_scope("ln2"):
            muf = rows.tile([1, TOK], f32, tag="row")
            nc.vector.tensor_scalar_mul(muf, sx2, 1.0 / D)
            m2 = rows.tile([1, TOK], f32, tag="row")
            nc.vector.tensor_scalar_mul(m2, sxx2, 1.0 / D)
            var = rows.tile([1, TOK], f32, tag="row")
            nc.vector.tensor_tensor(out=var, in0=muf, in1=muf, op=OP.mult)
            nc.vector.tensor_tensor(out=var, in0=m2, in1=var, op=OP.subtract)
            lnv = rows.tile([1, TOK], f32, tag="row")
            nc.scalar.activation(lnv, var, AF.Ln, bias=eps_sb[:])
            rstd = rows.tile([1, TOK], bf16, tag="rowb")
            with nc.allow_low_precision(reason="bf16 rstd broadcast"):
                nc.scalar.activation(rstd, lnv, AF.Exp, scale=-0.5)
            mub16 = rows.tile([1, TOK], bf16, tag="rowb")
            with nc.allow_low_precision(reason="bf16 mu broadcast"):
                nc.vector.tensor_copy(mub16, muf)
            mub = psum.tile([P, TOK], f32, tag="big", bufs=2, name="l2mub")
            nc.tensor.matmul(mub, ones_rb, mub16, start=True, stop=True)
            rsb = psum.tile([P, TOK], f32, tag="big", bufs=2, name="l2rsb")
            nc.tensor.matmul(rsb, ones_rb, rstd, start=True, stop=True)
            for k in range(KD):
                t1 = lnt.tile([P, TOK], bf16, tag="ln2")
                nc.vector.tensor_tensor(out=t1, in0=h1T[:, k, :], in1=mub,
                                        op=OP.subtract)
                nc.vector.tensor_tensor(out=mT[:, k, :], in0=t1, in1=rsb,
                                        op=OP.mult)

        # ---- phase 7: MLP ----
        hT = mlp.tile([P, KDI, TOK], bf16)
        with nc.named_scope("fc1"):
            for j in range(KDI):
                fwt = wgt.tile([P, KD, P], bf16, tag="fw", bufs=3)
                nc.sync.dma_start(fwt, fw[j])
                ps = psum.tile([P, TOK], f32, tag="big", bufs=2)
                for k in range(KD):
                    nc.tensor.matmul(ps, fwt[:, k, :], mT[:, k, :],
                                     start=(k == 0), stop=(k == KD - 1))
                nc.scalar.activation(hT[:, j, :], ps, AF.Gelu_apprx_tanh,
                                     bias=fb_sb[:, j:j + 1])
        with nc.named_scope("fc2"):
            for f in range(KD):
                gwt = wgt.tile([P, KDI, P], bf16, tag="gw", bufs=2)
                nc.sync.dma_start(gwt, gw[f])
                ps = psum.tile([P, TOK], f32, tag="big", bufs=2)
                for k in range(KDI):
                    nc.tensor.matmul(ps, gwt[:, k, :], hT[:, k, :],
                                     start=(k == 0), stop=(k == KDI - 1))
                o = outp.tile([P, TOK], f32, tag="ot")
                nc.vector.tensor_scalar_add(o, ps, gb_sb[:, f:f + 1])
                nc.vector.tensor_tensor(out=o, in0=o, in1=h1T[:, f, :],
                                        op=OP.add)
                nc.sync.dma_start(outT[P * f:P * (f + 1), :], o)

    nc.compile()
    return nc


def shard_inputs(inputs):
    """Full inputs -> list of 8 per-core input dicts (host-side layout only)."""
    bf16 = ml_dtypes.bfloat16
    f32 = np.float32
    hs = np.asarray(inputs["hidden_states"], f32).reshape(TT, D)
    l1g = np.asarray(inputs["ln1_g"], f32)
    l1b = np.asarray(inputs["ln1_b"], f32)
    l2g = np.asarray(inputs["ln2_g"], f32)
    l2b = np.asarray(inputs["ln2_b"], f32)
    # fold LN1 gamma/beta into attn_w/attn_b, LN2 into fc_w/fc_b
    attn_w = np.asarray(inputs["attn_w"], f32) * l1g[:, None]
    attn_b = np.asarray(inputs["attn_b"], f32) + l1b @ np.asarray(inputs["attn_w"], f32)
    fc_w = np.asarray(inputs["fc_w"], f32) * l2g[:, None]
    fc_b = np.asarray(inputs["fc_b"], f32) + l2b @ np.asarray(inputs["fc_w"], f32)

    def col(v):  # [D] -> [P, KD]
        return np.ascontiguousarray(np.asarray(v, f32).reshape(KD, P).T)

    pw = np.ascontiguousarray(np.asarray(inputs["proj_w"], f32)
                              .reshape(KD, P, KD, P).transpose(2, 1, 0, 3)
                              .astype(bf16))
    fw = np.ascontiguousarray(fc_w.reshape(KD, P, KDI, P).transpose(2, 1, 0, 3)
                              .astype(bf16))
    gw = np.ascontiguousarray(np.asarray(inputs["fc2_w"], f32)
                              .reshape(KDI, P, KD, P).transpose(2, 1, 0, 3)
                              .astype(bf16))
    pb = col(inputs["proj_b"])
    fbv = np.ascontiguousarray(fc_b.reshape(KDI, P).T)
    gbv = col(inputs["fc2_b"])

    ii, jj = np.meshgrid(np.arange(P), np.arange(QCH), indexing="ij")
    mkv = np.stack([(jj >= ii), (jj >= ii + P)]).astype(bf16)

    maps = []
    for c in range(NCORES):
        cols = np.r_[P * c:P * (c + 1),
                     D + P * c:D + P * (c + 1),
                     2 * D + P * c:2 * D + P * (c + 1)]
        aw_c = np.ascontiguousarray(
            attn_w[:, cols].reshape(KD, P, 3 * P).transpose(1, 0, 2)
            .astype(bf16))
        ab_c = np.ascontiguousarray(attn_b[cols], dtype=f32)
        xT_c = np.ascontiguousarray(
            hs[TOK * c:TOK * (c + 1)].T.reshape(KD, P, TOK).transpose(1, 0, 2)
        ).astype(bf16)
        maps.append({
            "xT": xT_c, "aw": aw_c, "ab": ab_c,
            "pw": pw, "pb": pb, "fw": fw, "fb": fbv, "gw": gw, "gb": gbv,
            "mk": mkv,
        })
    return maps


def unshard(results):
    out = np.concatenate([np.asarray(r["outT"]).T for r in results], axis=0)
    return np.ascontiguousarray(out.reshape(B, S, D))


def kernel(**inputs):
    global _CACHED_NC
    from concourse.bass_utils import run_bass_kernel_spmd
    if _CACHED_NC is None:
        _CACHED_NC = build_nc()
    in_maps = shard_inputs(inputs)
    res = run_bass_kernel_spmd(_CACHED_NC, in_maps,
                               core_ids=list(range(NCORES)))
    return unshard(res.results)


# revision 25
# speedup vs baseline: 1.0505x; 1.0250x over previous
"""Trainium2 Bass kernel for a GPT-2 style transformer block (nn_Block_16690242913196).

Sharding (8 NeuronCores, identical SPMD program):
  - LN1/QKV/proj/LN2/MLP: token-parallel (core i owns 512 flat tokens).
  - Attention: head-parallel (core i owns heads {2i, 2i+1}, all tokens).
  - Collective 1: AllGather of the RAW bf16 input (1MB/rank), triggered
    at t~0 straight from DRAM — no LN dependency, so it absorbs the
    cross-core start skew under useful work.
  - Collective 2: tiny AllGather of LN1 stats rows (mu, rstd; 2KB).
    Each consumer normalizes rank r's activations itself (2 broadcast
    matmuls + 16 bf16 DVE ops per rank, hidden in the QKV/attention era).
  - Collective 3: AllToAll of UNnormalized attention outputs plus
    RECIPROCAL softmax denominators (130 rows x 512 bf16); the divide
    becomes a broadcast-matmul + multiply on the token-parallel side.

  LN gamma/beta are folded into the consuming weights host-side. rstd
  is computed as exp(-0.5*ln(var+eps)) so LNs and the attention exp
  share one ACT table set. LN2 stats accumulate inside the proj loop.
"""

import numpy as np
import ml_dtypes

P = 128
B, S, D, H = 2, 2048, 1024, 16
DH = D // H          # 64
DI = 4 * D           # 4096
EPS = 1e-5
NCORES = 8
TT = B * S           # 4096 flat tokens
TOK = TT // NCORES   # 512 tokens per core
KD = D // P          # 8
KDI = DI // P        # 32
QCH = 256            # query chunk (2 blocks of 128)
NQC = S // QCH       # 8 query chunks per batch
HL = H // NCORES     # 2 local heads
RG = [list(range(NCORES))]

_CACHED_NC = None


def build_nc():
    import concourse.bacc as bacc
    import concourse.tile as tile
    import concourse.mybir as mybir
    from contextlib import ExitStack

    dt = mybir.dt
    f32, bf16, f32r = dt.float32, dt.bfloat16, dt.float32r
    f8 = dt.float8e4
    DR = mybir.MatmulPerfMode.DoubleRow
    AF = mybir.ActivationFunctionType
    OP = mybir.AluOpType

    nc = bacc.Bacc("TRN2", target_bir_lowering=False, debug=False,
                   num_devices=NCORES)

    # ---- kernel I/O (per-core shapes) ----
    xT = nc.dram_tensor("xT", [P, KD, TOK], bf16, kind="ExternalInput").ap()
    aw = nc.dram_tensor("aw", [P, KD, 3 * P], bf16, kind="ExternalInput").ap()
    ab = nc.dram_tensor("ab", [3 * P], f32, kind="ExternalInput").ap()
    pw = nc.dram_tensor("pw", [KD, P, KD, P], bf16, kind="ExternalInput").ap()
    pb = nc.dram_tensor("pb", [P, KD], f32, kind="ExternalInput").ap()
    fw = nc.dram_tensor("fw", [KDI, P, KD, P], bf16, kind="ExternalInput").ap()
    fb = nc.dram_tensor("fb", [P, KDI], f32, kind="ExternalInput").ap()
    gw = nc.dram_tensor("gw", [KD, P, KDI, P], bf16, kind="ExternalInput").ap()
    gb = nc.dram_tensor("gb", [P, KD], f32, kind="ExternalInput").ap()
    mk = nc.dram_tensor("mk", [2, P, QCH], bf16, kind="ExternalInput").ap()
    outT = nc.dram_tensor("outT", [D, TOK], f32, kind="ExternalOutput").ap()

    with tile.TileContext(nc) as tc, ExitStack() as ctx:
        const = ctx.enter_context(tc.tile_pool(name="const", bufs=1))
        dram = ctx.enter_context(tc.tile_pool(name="dram", bufs=1, space="DRAM"))
        psum = ctx.enter_context(tc.tile_pool(name="psum", bufs=1, space="PSUM"))
        rows = ctx.enter_context(tc.tile_pool(name="rows", bufs=6))
        sqp = ctx.enter_context(tc.tile_pool(name="sqp", bufs=2))
        lnt = ctx.enter_context(tc.tile_pool(name="lnt", bufs=3))
        dfp = ctx.enter_context(tc.tile_pool(name="dfp", bufs=8))
        res = ctx.enter_context(tc.tile_pool(name="res", bufs=1))

        # collective bounce buffers
        cc1_in = dram.tile([P, KD, TOK], bf16)
        cc1_out = dram.tile([NCORES, P, KD, TOK], bf16, addr_space="Shared")
        cc2_in = dram.tile([NCORES, P + 2, TOK], bf16)
        cc2_out = dram.tile([NCORES, P + 2, TOK], bf16)

        # input activations into SBUF (split per-k across DMA queues)
        xT_sb = res.tile([P, KD, TOK], bf16)
        for k in range(KD):
            nc.sync.dma_start(xT_sb[:, k, :], xT[:, k, :])

        # ---- constants in SBUF ----
        aw_sb = const.tile([P, KD, 3 * P], bf16)
        nc.sync.dma_start(aw_sb, aw)
        qbias = const.tile([P, 1], f32)
        nc.sync.dma_start(qbias, ab[0:P].rearrange("(p o) -> p o", o=1))
        kbias = const.tile([P, 1], f32)
        nc.sync.dma_start(kbias, ab[P:2 * P].rearrange("(p o) -> p o", o=1))
        vb_bc = const.tile([P, P], f32)
        nc.sync.dma_start(vb_bc, ab[2 * P:3 * P].rearrange("(o c) -> o c", o=1).to_broadcast((P, P)))
        mkc = const.tile([P, 2, QCH], bf16)
        nc.sync.dma_start(mkc, mk.rearrange("m p q -> p m q"))
        pb_sb = const.tile([P, KD], f32)
        nc.sync.dma_start(pb_sb, pb)
        fb_sb = const.tile([P, KDI], f32)
        nc.sync.dma_start(fb_sb, fb)
        gb_sb = const.tile([P, KD], f32)
        nc.sync.dma_start(gb_sb, gb)
        ones_cf = const.tile([P, 1], f32)
        nc.vector.memset(ones_cf, 1.0)
        ones_c = const.tile([P, 1], f32r)
        nc.vector.tensor_copy(ones_c, ones_cf)
        ones_cb = const.tile([P, 1], bf16)
        nc.vector.tensor_copy(ones_cb, ones_cf)
        ones_rf = const.tile([1, P], f32)
        nc.vector.memset(ones_rf, 1.0)
        ones_r = const.tile([1, P], f32r)
        nc.vector.tensor_copy(ones_r, ones_rf)
        ones_rb = const.tile([1, P], bf16)
        nc.vector.tensor_copy(ones_rb, ones_rf)
        eps_sb = const.tile([1, 1], f32)
        nc.vector.memset(eps_sb, EPS)

        # phase-scoped pools: attention-era tiles are freed before the MLP
        # pools (notably the 4MB hT) allocate.
        s1 = ExitStack()
        ares = s1.enter_context(tc.tile_pool(name="ares", bufs=1))
        xnp = s1.enter_context(tc.tile_pool(name="xnp", bufs=2))
        wp = s1.enter_context(tc.tile_pool(name="wp", bufs=8))
        psA = s1.enter_context(tc.tile_pool(name="psA", bufs=2, space="PSUM"))

        # ---- phase 1: LN1 stats only (full width) -> tiny stats AG ----
        with nc.named_scope("ln1"):
            sx = psum.tile([1, TOK], f32, tag="sm1", bufs=1, name="l1sx")
            for k in range(KD):
                nc.tensor.matmul(sx, ones_cb, xT_sb[:, k, :],
                                 start=(k == 0), stop=(k == KD - 1))
            sxx = psum.tile([1, TOK], f32, tag="sm2", bufs=1, name="l1sxx")
            for k in range(KD):
                sq = sqp.tile([P, TOK], bf16, tag="sq")
                nc.vector.tensor_mul(sq, xT_sb[:, k, :], xT_sb[:, k, :])
                nc.tensor.matmul(sxx, ones_cb, sq,
                                 start=(k == 0), stop=(k == KD - 1))
            muf = rows.tile([1, TOK], f32, tag="row")
            nc.vector.tensor_scalar_mul(muf, sx, 1.0 / D)
            m2 = rows.tile([1, TOK], f32, tag="row")
            nc.vector.tensor_scalar_mul(m2, sxx, 1.0 / D)
            var = rows.tile([1, TOK], f32, tag="row")
            nc.vector.tensor_tensor(out=var, in0=muf, in1=muf, op=OP.mult)
            nc.vector.tensor_tensor(out=var, in0=m2, in1=var, op=OP.subtract)
            lnv = rows.tile([1, TOK], f32, tag="row")
            nc.scalar.activation(lnv, var, AF.Ln, bias=eps_sb[:])
            rstd = rows.tile([1, TOK], bf16, tag="rowb")
            with nc.allow_low_precision(reason="bf16 rstd broadcast"):
                nc.scalar.activation(rstd, lnv, AF.Exp, scale=-0.5)
            mub16 = rows.tile([1, TOK], bf16, tag="rowb")
            with nc.allow_low_precision(reason="bf16 mu broadcast"):
                nc.vector.tensor_copy(mub16, muf)
            mub = psum.tile([P, TOK], f32, tag="big", bufs=2, name="l1mub")
            nc.tensor.matmul(mub, ones_rb, mub16, start=True, stop=True)
            rsb = psum.tile([P, TOK], f32, tag="big", bufs=2, name="l1rsb")
            nc.tensor.matmul(rsb, ones_rb, rstd, start=True, stop=True)
            xn = res.tile([P, KD, TOK], bf16)
            for k in range(KD):
                t1 = lnt.tile([P, TOK], bf16, tag="ln1")
                nc.vector.tensor_tensor(out=t1, in0=xT_sb[:, k, :], in1=mub,
                                        op=OP.subtract)
                nc.vector.tensor_tensor(out=xn[:, k, :], in0=t1, in1=rsb,
                                        op=OP.mult)
                nc.sync.dma_start(cc1_in[:, k, :], xn[:, k, :])
        with nc.named_scope("agx"):
            nc.gpsimd.collective_compute(
                "AllGather", OP.bypass, replica_groups=RG,
                ins=[cc1_in[:].opt()], outs=[cc1_out[:].opt()])

        # ---- phase 2+3: QKV (per rank) interleaved with attention ----
        kT = [ares.tile([P, TOK], bf16, name=f"kT{r}") for r in range(NCORES)]
        vA = [ares.tile([P, 4, HL, DH + 1], bf16, name=f"vA{r}")
              for r in range(NCORES)]
        qT = [ares.tile([P, TOK], bf16, name=f"qT{r}") for r in range(NCORES)]
        for r in range(NCORES):
            nc.vector.memset(vA[r][:, :, :, DH:DH + 1], 1.0)
        aT = ares.tile([P, NCORES, TOK], bf16)
        dT = ares.tile([1, HL, NCORES, TOK], bf16)
        dTr = ares.tile([1, HL, NCORES, TOK], bf16)

        def qkv_unit(r):
            with nc.named_scope(f"qkv{r}"):
                xn_r = xnp.tile([P, KD, TOK], bf16, tag="xnr")
                for kh in range(2):
                    nc.sync.dma_start(xn_r[:, 4 * kh:4 * (kh + 1), :],
                                      cc1_out[r][:, 4 * kh:4 * (kh + 1), :])
                for which in range(2):  # 0 -> q, 1 -> k
                    ps = psum.tile([P, TOK], f32, tag="big", bufs=2)
                    cb = which * P
                    for k in range(KD):
                        nc.tensor.matmul(ps, aw_sb[:, k, cb:cb + P],
                                         xn_r[:, k, :],
                                         start=(k == 0), stop=(k == KD - 1))
                    dst = qT[r] if which == 0 else kT[r]
                    bias = qbias if which == 0 else kbias
                    nc.scalar.activation(dst, ps, AF.Identity, bias=bias[:])
                for t in range(4):
                    psv = psum.tile([P, P], f32, tag="sm1" if t % 2 == 0
                                    else "sm2", bufs=1)
                    for k in range(KD):
                        nc.tensor.matmul(psv, xn_r[:, k, P * t:P * (t + 1)],
                                         aw_sb[:, k, 2 * P:3 * P],
                                         start=(k == 0), stop=(k == KD - 1))
                    nc.vector.tensor_tensor(
                        out=vA[r][:, t, :, 0:DH],
                        in0=psv.rearrange("p (h d) -> p h d", h=HL),
                        in1=vb_bc.rearrange("p (h d) -> p h d", h=HL),
                        op=OP.add)

        def attn_unit(b, qc):
            """Causal attention chunk: UNnormalized numerators into aT,
            reciprocal softmax denominators into dT (divide post-A2A).
            Software-pipelined: scores(pair p+1) overlap exp(pair p)."""
            qr = 4 * b + qc // 2
            qo = QCH * (qc % 2)
            nkb = 2 * qc + 2
            npair = nkb // 2
            accs = [psA.tile([DH + 1, QCH], f32, tag="acc", bufs=2,
                             name=f"acc{b}_{qc}_{h}")
                    for h in range(HL)]

            def emit_scores(p):
                kb0 = 2 * p
                ws = []
                for h in range(HL):
                    hb = DH * h
                    sc = psA.tile([P, 2 * QCH], f32, tag="sc", bufs=2)
                    for j in range(2):
                        kb = kb0 + j
                        r = 4 * b + kb // 4
                        t = kb % 4
                        nc.tensor.matmul(
                            sc[:, QCH * j:QCH * (j + 1)],
                            kT[r][hb:hb + DH, P * t:P * (t + 1)],
                            qT[qr][hb:hb + DH, qo:qo + QCH],
                            start=True, stop=True,
                            skip_group_check=True)
                    w = wp.tile([P, 2 * QCH], bf16, tag="w")
                    nc.scalar.activation(w, sc, AF.Exp, scale=0.125)
                    if kb0 == 2 * qc:  # diagonal pair: apply masks
                        nc.vector.tensor_mul(
                            w.rearrange("p (m q) -> p m q", m=2),
                            w.rearrange("p (m q) -> p m q", m=2),
                            mkc)
                    ws.append(w)
                return ws

            def emit_av(p, ws):
                kb0 = 2 * p
                for h in range(HL):
                    for j in range(2):
                        kb = kb0 + j
                        r = 4 * b + kb // 4
                        t = kb % 4
                        nc.tensor.matmul(
                            accs[h], vA[r][:, t, h, :],
                            ws[h][:, QCH * j:QCH * (j + 1)],
                            start=(kb == 0), stop=(kb == nkb - 1),
                            skip_group_check=True)

            ws_p = emit_scores(0)
            for p in range(npair):
                ws_n = emit_scores(p + 1) if p + 1 < npair else None
                emit_av(p, ws_p)
                ws_p = ws_n
            for h in range(HL):
                hb = DH * h
                nc.vector.tensor_copy(aT[hb:hb + DH, qr, qo:qo + QCH],
                                      accs[h][0:DH, :])
                nc.vector.tensor_copy(dT[0:1, h, qr, qo:qo + QCH],
                                      accs[h][DH:DH + 1, :])
            if qc % 2 == 1:  # chunk qr complete: recip + stage its slot
                for h in range(HL):
                    with nc.allow_low_precision(reason="bf16 softmax denom"):
                        nc.vector.reciprocal(dTr[0:1, h, qr, :],
                                             dT[0:1, h, qr, :])
                nc.sync.dma_start(cc2_in[qr, 0:P], aT[:, qr, :])
                nc.sync.dma_start(cc2_in[qr, P:P + 2], dTr[0:1, :, qr, :])

        with nc.named_scope("qkv_attn"):
            qkv_unit(0)
            for b in range(B):
                for qc in range(NQC):
                    attn_unit(b, qc)
                    if qc % 2 == 0:
                        r = 4 * b + qc // 2 + 1
                        if r < NCORES:
                            qkv_unit(r)

        # ---- phase 4: AllToAll back to token-parallel ----
        with nc.named_scope("a2a"):
            nc.gpsimd.collective_compute(
                "AllToAll", OP.bypass, replica_groups=RG,
                ins=[cc2_in[:].opt()], outs=[cc2_out[:].opt()])
        # latency-critical result loads: emit BEFORE the MLP weight
        # avalanche (s1.close frees SBUF and unleashes wgt prefetch DMAs)
        aF = [dfp.tile([P, TOK], bf16, tag="aF", name=f"aF{r}")
              for r in range(NCORES)]
        aFn = [dfp.tile([P, TOK], bf16, tag="aFn", name=f"aFn{r}")
               for r in range(NCORES)]
        dfh = [dfp.tile([1, 2 * TOK], bf16, tag="dfh", name=f"dfh{r}")
               for r in range(NCORES)]
        with nc.named_scope("resload"):
            for r in range(NCORES):
                nc.sync.dma_start(aF[r], cc2_out[r, 0:P])
                nc.sync.dma_start(dfh[r][:, 0:TOK], cc2_out[r, P:P + 1])
                nc.sync.dma_start(dfh[r][:, TOK:2 * TOK],
                                  cc2_out[r, P + 1:P + 2])

        s1.close()  # release attention-era SBUF
        psB = ctx.enter_context(tc.tile_pool(name="psB", bufs=4, space="PSUM"))
        mlp = ctx.enter_context(tc.tile_pool(name="mlp", bufs=1))
        wgt = ctx.enter_context(tc.tile_pool(name="wgt", bufs=1))
        outp = ctx.enter_context(tc.tile_pool(name="outp", bufs=2))

        # ---- phase 5: softmax normalize + output projection + residual ----
        h1T = mlp.tile([P, KD, TOK], bf16)
        with nc.named_scope("proj"):
            for r in range(NCORES):
                rb0 = psB.tile([DH, TOK], f32, tag="rb",
                               bufs=4, name=f"rb0_{r}")
                nc.tensor.matmul(rb0, ones_rb[:, 0:DH], dfh[r][:, 0:TOK],
                                 start=True, stop=True, skip_group_check=True)
                rb1 = psB.tile([DH, TOK], f32, tag="rb",
                               bufs=4, name=f"rb1_{r}")
                nc.tensor.matmul(rb1, ones_rb[:, 0:DH], dfh[r][:, TOK:],
                                 start=True, stop=True, skip_group_check=True)
                nc.vector.tensor_mul(aFn[r][0:DH, :], aF[r][0:DH, :], rb0)
                nc.vector.tensor_mul(aFn[r][DH:P, :], aF[r][DH:P, :], rb1)
            # proj matmuls + incremental LN2 stats (chained across f)
            sx2 = psum.tile([1, TOK], f32, tag="sm1", bufs=1, name="l2sx")
            sxx2 = psum.tile([1, TOK], f32, tag="sm2", bufs=1, name="l2sxx")
            for f in range(KD):
                pwt = wgt.tile([P, KD, P], bf16, tag="pw", bufs=2)
                nc.sync.dma_start(pwt, pw[f])
                ps = psum.tile([P, TOK], f32, tag="big", bufs=2)
                for k in range(KD):
                    nc.tensor.matmul(ps, pwt[:, k, :], aFn[k],
                                     start=(k == 0), stop=(k == KD - 1))
                t1 = lnt.tile([P, TOK], f32, tag="pj")
                nc.vector.tensor_scalar_add(t1, ps, pb_sb[:, f:f + 1])
                nc.vector.tensor_tensor(out=h1T[:, f, :], in0=t1,
                                        in1=xT_sb[:, f, :], op=OP.add)
                nc.tensor.matmul(sx2, ones_cb, h1T[:, f, :],
                                 start=(f == 0), stop=(f == KD - 1))
                sq2 = sqp.tile([P, TOK], bf16, tag="sq2")
                nc.vector.tensor_mul(sq2, h1T[:, f, :], h1T[:, f, :])
                nc.tensor.matmul(sxx2, ones_cb, sq2,
                                 start=(f == 0), stop=(f == KD - 1))

        # ---- phase 6: LN2 finish (stats already accumulated) ----
        mT = mlp.tile([P, KD, TOK], bf16)
        with nc.named_scope("ln2"):
            muf = rows.tile([1, TOK], f32, tag="row")
            nc.vector.tensor_scalar_mul(muf, sx2, 1.0 / D)
            m2 = rows.tile([1, TOK], f32, tag="row")
            nc.vector.tensor_scalar_mul(m2, sxx2, 1.0 / D)
            var = rows.tile([1, TOK], f32, tag="row")
            nc.vector.tensor_tensor(out=var, in0=muf, in1=muf, op=OP.mult)
            nc.vector.tensor_tensor(out=var, in0=m2, in1=var, op=OP.subtract)
            lnv = rows.tile([1, TOK], f32, tag="row")
            nc.scalar.activation(lnv, var, AF.Ln, bias=eps_sb[:])
            rstd = rows.tile([1, TOK], bf16, tag="rowb")
            with nc.allow_low_precision(reason="bf16 rstd broadcast"):
                nc.scalar.activation(rstd, lnv, AF.Exp, scale=-0.5)
            mub16 = rows.tile([1, TOK], bf16, tag="rowb")
            with nc.allow_low_precision(reason="bf16 mu broadcast"):
                nc.vector.tensor_copy(mub16, muf)
            mub = psum.tile([P, TOK], f32, tag="big", bufs=2, name="l2mub")
            nc.tensor.matmul(mub, ones_rb, mub16, start=True, stop=True)
            rsb = psum.tile([P, TOK], f32, tag="big", bufs=2, name="l2rsb")
            nc.tensor.matmul(rsb, ones_rb, rstd, start=True, stop=True)
            mubs = lnt.tile([P, TOK], bf16, tag="l2mb")
            nc.vector.tensor_copy(mubs, mub)
            rsbs = lnt.tile([P, TOK], bf16, tag="l2rb")
            nc.vector.tensor_copy(rsbs, rsb)
            for k in range(KD):
                t1 = lnt.tile([P, TOK], bf16, tag="ln2")
                nc.vector.tensor_tensor(out=t1, in0=h1T[:, k, :], in1=mubs,
                                        op=OP.subtract)
                nc.vector.tensor_tensor(out=mT[:, k, :], in0=t1, in1=rsbs,
                                        op=OP.mult)

        # ---- phase 7: MLP ----
        hT = mlp.tile([P, KDI, TOK], bf16)
        with nc.named_scope("fc1"):
            for j in range(KDI):
                fwt = wgt.tile([P, KD, P], bf16, tag="fw", bufs=3)
                nc.sync.dma_start(fwt, fw[j])
                ps = psum.tile([P, TOK], f32, tag="big", bufs=2)
                for k in range(KD):
                    nc.tensor.matmul(ps, fwt[:, k, :], mT[:, k, :],
                                     start=(k == 0), stop=(k == KD - 1))
                nc.scalar.activation(hT[:, j, :], ps, AF.Gelu_apprx_tanh,
                                     bias=fb_sb[:, j:j + 1])
        with nc.named_scope("fc2"):
            for f in range(KD):
                gwt = wgt.tile([P, KDI, P], bf16, tag="gw", bufs=2)
                nc.sync.dma_start(gwt, gw[f])
                ps = psum.tile([P, TOK], f32, tag="big", bufs=2)
                for k in range(KDI):
                    nc.tensor.matmul(ps, gwt[:, k, :], hT[:, k, :],
                                     start=(k == 0), stop=(k == KDI - 1))
                o = outp.tile([P, TOK], f32, tag="ot")
                nc.vector.tensor_scalar_add(o, ps, gb_sb[:, f:f + 1])
                nc.vector.tensor_tensor(out=o, in0=o, in1=h1T[:, f, :],
                                        op=OP.add)
                nc.sync.dma_start(outT[P * f:P * (f + 1), :], o)

    nc.compile()
    return nc


def shard_inputs(inputs):
    """Full inputs -> list of 8 per-core input dicts (host-side layout only)."""
    bf16 = ml_dtypes.bfloat16
    f32 = np.float32
    hs = np.asarray(inputs["hidden_states"], f32).reshape(TT, D)
    l1g = np.asarray(inputs["ln1_g"], f32)
    l1b = np.asarray(inputs["ln1_b"], f32)
    l2g = np.asarray(inputs["ln2_g"], f32)
    l2b = np.asarray(inputs["ln2_b"], f32)
    # fold LN1 gamma/beta into attn_w/attn_b, LN2 into fc_w/fc_b
    attn_w = np.asarray(inputs["attn_w"], f32) * l1g[:, None]
    attn_b = np.asarray(inputs["attn_b"], f32) + l1b @ np.asarray(inputs["attn_w"], f32)
    fc_w = np.asarray(inputs["fc_w"], f32) * l2g[:, None]
    fc_b = np.asarray(inputs["fc_b"], f32) + l2b @ np.asarray(inputs["fc_w"], f32)

    def col(v):  # [D] -> [P, KD]
        return np.ascontiguousarray(np.asarray(v, f32).reshape(KD, P).T)

    pw = np.ascontiguousarray(np.asarray(inputs["proj_w"], f32)
                              .reshape(KD, P, KD, P).transpose(2, 1, 0, 3)
                              .astype(bf16))
    fw = np.ascontiguousarray(fc_w.reshape(KD, P, KDI, P).transpose(2, 1, 0, 3)
                              .astype(bf16))
    gw = np.ascontiguousarray(np.asarray(inputs["fc2_w"], f32)
                              .reshape(KDI, P, KD, P).transpose(2, 1, 0, 3)
                              .astype(bf16))
    pb = col(inputs["proj_b"])
    fbv = np.ascontiguousarray(fc_b.reshape(KDI, P).T)
    gbv = col(inputs["fc2_b"])

    ii, jj = np.meshgrid(np.arange(P), np.arange(QCH), indexing="ij")
    mkv = np.stack([(jj >= ii), (jj >= ii + P)]).astype(bf16)

    maps = []
    for c in range(NCORES):
        cols = np.r_[P * c:P * (c + 1),
                     D + P * c:D + P * (c + 1),
                     2 * D + P * c:2 * D + P * (c + 1)]
        aw_c = np.ascontiguousarray(
            attn_w[:, cols].reshape(KD, P, 3 * P).transpose(1, 0, 2)
            .astype(bf16))
        ab_c = np.ascontiguousarray(attn_b[cols], dtype=f32)
        xT_c = np.ascontiguousarray(
            hs[TOK * c:TOK * (c + 1)].T.reshape(KD, P, TOK).transpose(1, 0, 2)
        ).astype(bf16)
        maps.append({
            "xT": xT_c, "aw": aw_c, "ab": ab_c,
            "pw": pw, "pb": pb, "fw": fw, "fb": fbv, "gw": gw, "gb": gbv,
            "mk": mkv,
        })
    return maps


def unshard(results):
    out = np.concatenate([np.asarray(r["outT"]).T for r in results], axis=0)
    return np.ascontiguousarray(out.reshape(B, S, D))


def kernel(**inputs):
    global _CACHED_NC
    from concourse.bass_utils import run_bass_kernel_spmd
    if _CACHED_NC is None:
        _CACHED_NC = build_nc()
    in_maps = shard_inputs(inputs)
    res = run_bass_kernel_spmd(_CACHED_NC, in_maps,
                               core_ids=list(range(NCORES)))
    return unshard(res.results)


# revision 27
# speedup vs baseline: 1.0618x; 1.0107x over previous
"""Trainium2 Bass kernel for a GPT-2 style transformer block (nn_Block_16690242913196).

Sharding (8 NeuronCores, identical SPMD program):
  - LN1/QKV/proj/LN2/MLP: token-parallel (core i owns 512 flat tokens).
  - Attention: head-parallel (core i owns heads {2i, 2i+1}, all tokens).
  - Collective 1: AllGather of the RAW bf16 input (1MB/rank), triggered
    at t~0 straight from DRAM — no LN dependency, so it absorbs the
    cross-core start skew under useful work.
  - Collective 2: tiny AllGather of LN1 stats rows (mu, rstd; 2KB).
    Each consumer normalizes rank r's activations itself (2 broadcast
    matmuls + 16 bf16 DVE ops per rank, hidden in the QKV/attention era).
  - Collective 3: AllToAll of UNnormalized attention outputs plus
    RECIPROCAL softmax denominators (130 rows x 512 bf16); the divide
    becomes a broadcast-matmul + multiply on the token-parallel side.

  LN gamma/beta are folded into the consuming weights host-side. rstd
  is computed as exp(-0.5*ln(var+eps)) so LNs and the attention exp
  share one ACT table set. LN2 stats accumulate inside the proj loop.
"""

import numpy as np
import ml_dtypes

P = 128
B, S, D, H = 2, 2048, 1024, 16
DH = D // H          # 64
DI = 4 * D           # 4096
EPS = 1e-5
NCORES = 8
TT = B * S           # 4096 flat tokens
TOK = TT // NCORES   # 512 tokens per core
KD = D // P          # 8
KDI = DI // P        # 32
QCH = 256            # query chunk (2 blocks of 128)
NQC = S // QCH       # 8 query chunks per batch
HL = H // NCORES     # 2 local heads
RG = [list(range(NCORES))]

_CACHED_NC = None


def build_nc():
    import concourse.bacc as bacc
    import concourse.tile as tile
    import concourse.mybir as mybir
    from contextlib import ExitStack

    dt = mybir.dt
    f32, bf16, f32r = dt.float32, dt.bfloat16, dt.float32r
    f8 = dt.float8e4
    DR = mybir.MatmulPerfMode.DoubleRow
    AF = mybir.ActivationFunctionType
    OP = mybir.AluOpType

    nc = bacc.Bacc("TRN2", target_bir_lowering=False, debug=False,
                   num_devices=NCORES)

    # ---- kernel I/O (per-core shapes) ----
    xT = nc.dram_tensor("xT", [P, KD, TOK], bf16, kind="ExternalInput").ap()
    aw = nc.dram_tensor("aw", [P, KD, 3 * P], bf16, kind="ExternalInput").ap()
    ab = nc.dram_tensor("ab", [3 * P], f32, kind="ExternalInput").ap()
    pw = nc.dram_tensor("pw", [KD, P, KD, P], bf16, kind="ExternalInput").ap()
    pb = nc.dram_tensor("pb", [P, KD], f32, kind="ExternalInput").ap()
    fw = nc.dram_tensor("fw", [KDI, P, KD, P], bf16, kind="ExternalInput").ap()
    fb = nc.dram_tensor("fb", [P, KDI], f32, kind="ExternalInput").ap()
    gw = nc.dram_tensor("gw", [KD, P, KDI, P], bf16, kind="ExternalInput").ap()
    gb = nc.dram_tensor("gb", [P, KD], f32, kind="ExternalInput").ap()
    mk = nc.dram_tensor("mk", [2, P, QCH], bf16, kind="ExternalInput").ap()
    outT = nc.dram_tensor("outT", [D, TOK], f32, kind="ExternalOutput").ap()

    with tile.TileContext(nc) as tc, ExitStack() as ctx:
        const = ctx.enter_context(tc.tile_pool(name="const", bufs=1))
        dram = ctx.enter_context(tc.tile_pool(name="dram", bufs=1, space="DRAM"))
        psum = ctx.enter_context(tc.tile_pool(name="psum", bufs=1, space="PSUM"))
        rows = ctx.enter_context(tc.tile_pool(name="rows", bufs=6))
        sqp = ctx.enter_context(tc.tile_pool(name="sqp", bufs=2))
        lnt = ctx.enter_context(tc.tile_pool(name="lnt", bufs=3))
        dfp = ctx.enter_context(tc.tile_pool(name="dfp", bufs=8))
        res = ctx.enter_context(tc.tile_pool(name="res", bufs=1))

        # collective bounce buffers
        cc1_in = dram.tile([P, KD, TOK], bf16)
        cc1_out = dram.tile([NCORES, P, KD, TOK], bf16, addr_space="Shared")
        cc2_in = dram.tile([NCORES, P + 2, TOK], bf16)
        cc2_out = dram.tile([NCORES, P + 2, TOK], bf16)

        # input activations into SBUF (split per-k across DMA queues)
        xT_sb = res.tile([P, KD, TOK], bf16)
        for k in range(KD):
            nc.sync.dma_start(xT_sb[:, k, :], xT[:, k, :])

        # ---- constants in SBUF ----
        aw_sb = const.tile([P, KD, 3 * P], bf16)
        nc.sync.dma_start(aw_sb, aw)
        qbias = const.tile([P, 1], f32)
        nc.sync.dma_start(qbias, ab[0:P].rearrange("(p o) -> p o", o=1))
        kbias = const.tile([P, 1], f32)
        nc.sync.dma_start(kbias, ab[P:2 * P].rearrange("(p o) -> p o", o=1))
        vb_bc = const.tile([P, P], f32)
        nc.sync.dma_start(vb_bc, ab[2 * P:3 * P].rearrange("(o c) -> o c", o=1).to_broadcast((P, P)))
        mkc = const.tile([P, 2, QCH], bf16)
        nc.sync.dma_start(mkc, mk.rearrange("m p q -> p m q"))
        pb_sb = const.tile([P, KD], f32)
        nc.sync.dma_start(pb_sb, pb)
        fb_sb = const.tile([P, KDI], f32)
        nc.sync.dma_start(fb_sb, fb)
        gb_sb = const.tile([P, KD], f32)
        nc.sync.dma_start(gb_sb, gb)
        ones_cf = const.tile([P, 1], f32)
        nc.vector.memset(ones_cf, 1.0)
        ones_c = const.tile([P, 1], f32r)
        nc.vector.tensor_copy(ones_c, ones_cf)
        ones_cb = const.tile([P, 1], bf16)
        nc.vector.tensor_copy(ones_cb, ones_cf)
        ones_rf = const.tile([1, P], f32)
        nc.vector.memset(ones_rf, 1.0)
        ones_r = const.tile([1, P], f32r)
        nc.vector.tensor_copy(ones_r, ones_rf)
        ones_rb = const.tile([1, P], bf16)
        nc.vector.tensor_copy(ones_rb, ones_rf)
        eps_sb = const.tile([1, 1], f32)
        nc.vector.memset(eps_sb, EPS)

        # phase-scoped pools: attention-era tiles are freed before the MLP
        # pools (notably the 4MB hT) allocate.
        s1 = ExitStack()
        ares = s1.enter_context(tc.tile_pool(name="ares", bufs=1))
        xnp = s1.enter_context(tc.tile_pool(name="xnp", bufs=2))
        wp = s1.enter_context(tc.tile_pool(name="wp", bufs=8))
        psA = s1.enter_context(tc.tile_pool(name="psA", bufs=2, space="PSUM"))

        # ---- phase 1: LN1 stats only (full width) -> tiny stats AG ----
        with nc.named_scope("ln1"):
            sx = psum.tile([1, TOK], f32, tag="sm1", bufs=1, name="l1sx")
            for k in range(KD):
                nc.tensor.matmul(sx, ones_cb, xT_sb[:, k, :],
                                 start=(k == 0), stop=(k == KD - 1))
            sxx = psum.tile([1, TOK], f32, tag="sm2", bufs=1, name="l1sxx")
            for k in range(KD):
                sq = sqp.tile([P, TOK], bf16, tag="sq")
                nc.vector.tensor_mul(sq, xT_sb[:, k, :], xT_sb[:, k, :])
                nc.tensor.matmul(sxx, ones_cb, sq,
                                 start=(k == 0), stop=(k == KD - 1))
            muf = rows.tile([1, TOK], f32, tag="row")
            nc.vector.tensor_scalar_mul(muf, sx, 1.0 / D)
            m2 = rows.tile([1, TOK], f32, tag="row")
            nc.vector.tensor_scalar_mul(m2, sxx, 1.0 / D)
            var = rows.tile([1, TOK], f32, tag="row")
            nc.vector.tensor_tensor(out=var, in0=muf, in1=muf, op=OP.mult)
            nc.vector.tensor_tensor(out=var, in0=m2, in1=var, op=OP.subtract)
            lnv = rows.tile([1, TOK], f32, tag="row")
            nc.scalar.activation(lnv, var, AF.Ln, bias=eps_sb[:])
            rstd = rows.tile([1, TOK], bf16, tag="rowb")
            with nc.allow_low_precision(reason="bf16 rstd broadcast"):
                nc.scalar.activation(rstd, lnv, AF.Exp, scale=-0.5)
            mub16 = rows.tile([1, TOK], bf16, tag="rowb")
            with nc.allow_low_precision(reason="bf16 mu broadcast"):
                nc.vector.tensor_copy(mub16, muf)
            mub = psum.tile([P, TOK], f32, tag="big", bufs=2, name="l1mub")
            nc.tensor.matmul(mub, ones_rb, mub16, start=True, stop=True)
            rsb = psum.tile([P, TOK], f32, tag="big", bufs=2, name="l1rsb")
            nc.tensor.matmul(rsb, ones_rb, rstd, start=True, stop=True)
            xn = res.tile([P, KD, TOK], bf16)
            for k in range(KD):
                t1 = lnt.tile([P, TOK], bf16, tag="ln1")
                nc.vector.tensor_tensor(out=t1, in0=xT_sb[:, k, :], in1=mub,
                                        op=OP.subtract)
                nc.vector.tensor_tensor(out=xn[:, k, :], in0=t1, in1=rsb,
                                        op=OP.mult)
                nc.sync.dma_start(cc1_in[:, k, :], xn[:, k, :])
        with nc.named_scope("agx"):
            nc.gpsimd.collective_compute(
                "AllGather", OP.bypass, replica_groups=RG,
                ins=[cc1_in[:].opt()], outs=[cc1_out[:].opt()])

        # ---- phase 2+3: QKV (per rank) interleaved with attention ----
        kT = [ares.tile([P, TOK], bf16, name=f"kT{r}") for r in range(NCORES)]
        vA = [ares.tile([P, 4, HL, DH + 1], bf16, name=f"vA{r}")
              for r in range(NCORES)]
        qT = [ares.tile([P, TOK], bf16, name=f"qT{r}") for r in range(NCORES)]
        for r in range(NCORES):
            nc.vector.memset(vA[r][:, :, :, DH:DH + 1], 1.0)
        aT = ares.tile([P, NCORES, TOK], bf16)
        dT = ares.tile([1, HL, NCORES, TOK], bf16)
        dTr = ares.tile([1, HL, NCORES, TOK], bf16)

        def qkv_unit(r):
            with nc.named_scope(f"qkv{r}"):
                xn_r = xnp.tile([P, KD, TOK], bf16, tag="xnr")
                for kh in range(2):
                    nc.sync.dma_start(xn_r[:, 4 * kh:4 * (kh + 1), :],
                                      cc1_out[r][:, 4 * kh:4 * (kh + 1), :])
                for which in range(2):  # 0 -> q, 1 -> k
                    ps = psum.tile([P, TOK], f32, tag="big", bufs=2)
                    cb = which * P
                    for k in range(KD):
                        nc.tensor.matmul(ps, aw_sb[:, k, cb:cb + P],
                                         xn_r[:, k, :],
                                         start=(k == 0), stop=(k == KD - 1))
                    dst = qT[r] if which == 0 else kT[r]
                    bias = qbias if which == 0 else kbias
                    nc.scalar.activation(dst, ps, AF.Identity, bias=bias[:])
                for t in range(4):
                    psv = psum.tile([P, P], f32, tag="sm1" if t % 2 == 0
                                    else "sm2", bufs=1)
                    for k in range(KD):
                        nc.tensor.matmul(psv, xn_r[:, k, P * t:P * (t + 1)],
                                         aw_sb[:, k, 2 * P:3 * P],
                                         start=(k == 0), stop=(k == KD - 1))
                    nc.vector.tensor_tensor(
                        out=vA[r][:, t, :, 0:DH],
                        in0=psv.rearrange("p (h d) -> p h d", h=HL),
                        in1=vb_bc.rearrange("p (h d) -> p h d", h=HL),
                        op=OP.add)

        def attn_unit(b, qc):
            """Causal attention chunk: UNnormalized numerators into aT,
            reciprocal softmax denominators into dT (divide post-A2A).
            Software-pipelined: scores(pair p+1) overlap exp(pair p)."""
            qr = 4 * b + qc // 2
            qo = QCH * (qc % 2)
            nkb = 2 * qc + 2
            npair = nkb // 2
            accs = [psA.tile([DH + 1, QCH], f32, tag="acc", bufs=2,
                             name=f"acc{b}_{qc}_{h}")
                    for h in range(HL)]

            def emit_scores(p):
                kb0 = 2 * p
                ws = []
                for h in range(HL):
                    hb = DH * h
                    sc = psA.tile([P, 2 * QCH], f32, tag="sc", bufs=2)
                    for j in range(2):
                        kb = kb0 + j
                        r = 4 * b + kb // 4
                        t = kb % 4
                        nc.tensor.matmul(
                            sc[:, QCH * j:QCH * (j + 1)],
                            kT[r][hb:hb + DH, P * t:P * (t + 1)],
                            qT[qr][hb:hb + DH, qo:qo + QCH],
                            start=True, stop=True,
                            skip_group_check=True)
                    w = wp.tile([P, 2 * QCH], bf16, tag="w")
                    nc.scalar.activation(w, sc, AF.Exp, scale=0.125)
                    if kb0 == 2 * qc:  # diagonal pair: apply masks
                        nc.vector.tensor_mul(
                            w.rearrange("p (m q) -> p m q", m=2),
                            w.rearrange("p (m q) -> p m q", m=2),
                            mkc)
                    ws.append(w)
                return ws

            def emit_av(p, ws):
                kb0 = 2 * p
                for h in range(HL):
                    for j in range(2):
                        kb = kb0 + j
                        r = 4 * b + kb // 4
                        t = kb % 4
                        nc.tensor.matmul(
                            accs[h], vA[r][:, t, h, :],
                            ws[h][:, QCH * j:QCH * (j + 1)],
                            start=(kb == 0), stop=(kb == nkb - 1),
                            skip_group_check=True)

            ws_p = emit_scores(0)
            for p in range(npair):
                ws_n = emit_scores(p + 1) if p + 1 < npair else None
                emit_av(p, ws_p)
                ws_p = ws_n
            for h in range(HL):
                hb = DH * h
                nc.vector.tensor_copy(aT[hb:hb + DH, qr, qo:qo + QCH],
                                      accs[h][0:DH, :])
                nc.vector.tensor_copy(dT[0:1, h, qr, qo:qo + QCH],
                                      accs[h][DH:DH + 1, :])
            if qc % 2 == 1:  # chunk qr complete: recip + stage its slot
                for h in range(HL):
                    with nc.allow_low_precision(reason="bf16 softmax denom"):
                        nc.vector.reciprocal(dTr[0:1, h, qr, :],
                                             dT[0:1, h, qr, :])
                nc.sync.dma_start(cc2_in[qr, 0:P], aT[:, qr, :])
                nc.sync.dma_start(cc2_in[qr, P:P + 2], dTr[0:1, :, qr, :])

        with nc.named_scope("qkv_attn"):
            qkv_unit(0)
            for b in range(B):
                for qc in range(NQC):
                    attn_unit(b, qc)
                    if qc % 2 == 0:
                        r = 4 * b + qc // 2 + 1
                        if r < NCORES:
                            qkv_unit(r)

        # ---- phase 4: AllToAll back to token-parallel ----
        with nc.named_scope("a2a"):
            nc.gpsimd.collective_compute(
                "AllToAll", OP.bypass, replica_groups=RG,
                ins=[cc2_in[:].opt()], outs=[cc2_out[:].opt()])
        # latency-critical result loads: emit BEFORE the MLP weight
        # avalanche (s1.close frees SBUF and unleashes wgt prefetch DMAs)
        aF = [dfp.tile([P, TOK], bf16, tag="aF", name=f"aF{r}")
              for r in range(NCORES)]
        aFn = [dfp.tile([P, TOK], bf16, tag="aFn", name=f"aFn{r}")
               for r in range(NCORES)]
        dfh = [dfp.tile([1, 2 * TOK], bf16, tag="dfh", name=f"dfh{r}")
               for r in range(NCORES)]
        with nc.named_scope("resload"):
            for r in range(NCORES):
                nc.sync.dma_start(aF[r], cc2_out[r, 0:P])
                nc.sync.dma_start(dfh[r][:, 0:TOK], cc2_out[r, P:P + 1])
                nc.sync.dma_start(dfh[r][:, TOK:2 * TOK],
                                  cc2_out[r, P + 1:P + 2])

        s1.close()  # release attention-era SBUF
        psB = ctx.enter_context(tc.tile_pool(name="psB", bufs=4, space="PSUM"))
        mlp = ctx.enter_context(tc.tile_pool(name="mlp", bufs=1))
        wgt = ctx.enter_context(tc.tile_pool(name="wgt", bufs=1))
        outp = ctx.enter_context(tc.tile_pool(name="outp", bufs=2))

        # ---- phase 5: softmax normalize + output projection + residual ----
        h1T = mlp.tile([P, KD, TOK], bf16)
        with nc.named_scope("proj"):
            for r in range(NCORES):
                rb0 = psB.tile([DH, TOK], f32, tag="rb",
                               bufs=4, name=f"rb0_{r}")
                nc.tensor.matmul(rb0, ones_rb[:, 0:DH], dfh[r][:, 0:TOK],
                                 start=True, stop=True, skip_group_check=True)
                rb1 = psB.tile([DH, TOK], f32, tag="rb",
                               bufs=4, name=f"rb1_{r}")
                nc.tensor.matmul(rb1, ones_rb[:, 0:DH], dfh[r][:, TOK:],
                                 start=True, stop=True, skip_group_check=True)
                nc.vector.tensor_mul(aFn[r][0:DH, :], aF[r][0:DH, :], rb0)
                nc.vector.tensor_mul(aFn[r][DH:P, :], aF[r][DH:P, :], rb1)
            # proj matmuls + incremental LN2 stats (chained across f)
            sx2 = psum.tile([1, TOK], f32, tag="sm1", bufs=1, name="l2sx")
            sxx2 = psum.tile([1, TOK], f32, tag="sm2", bufs=1, name="l2sxx")
            for f in range(KD):
                pwt = wgt.tile([P, KD, P], bf16, tag="pw", bufs=2)
                nc.sync.dma_start(pwt, pw[f])
                ps = psum.tile([P, TOK], f32, tag="big", bufs=2)
                for k in range(KD):
                    nc.tensor.matmul(ps, pwt[:, k, :], aFn[k],
                                     start=(k == 0), stop=(k == KD - 1))
                t1 = lnt.tile([P, TOK], f32, tag="pj")
                nc.vector.tensor_scalar_add(t1, ps, pb_sb[:, f:f + 1])
                nc.vector.tensor_tensor(out=h1T[:, f, :], in0=t1,
                                        in1=xT_sb[:, f, :], op=OP.add)
                nc.tensor.matmul(sx2, ones_cb, h1T[:, f, :],
                                 start=(f == 0), stop=(f == KD - 1))
                sq2 = sqp.tile([P, TOK], bf16, tag="sq2")
                nc.vector.tensor_mul(sq2, h1T[:, f, :], h1T[:, f, :])
                nc.tensor.matmul(sxx2, ones_cb, sq2,
                                 start=(f == 0), stop=(f == KD - 1))

        # ---- phase 6: LN2 finish (stats already accumulated) ----
        mT = mlp.tile([P, KD, TOK], bf16)
        with nc.named_scope("ln2"):
            muf = rows.tile([1, TOK], f32, tag="row")
            nc.vector.tensor_scalar_mul(muf, sx2, 1.0 / D)
            m2 = rows.tile([1, TOK], f32, tag="row")
            nc.vector.tensor_scalar_mul(m2, sxx2, 1.0 / D)
            var = rows.tile([1, TOK], f32, tag="row")
            nc.vector.tensor_tensor(out=var, in0=muf, in1=muf, op=OP.mult)
            nc.vector.tensor_tensor(out=var, in0=m2, in1=var, op=OP.subtract)
            lnv = rows.tile([1, TOK], f32, tag="row")
            nc.scalar.activation(lnv, var, AF.Ln, bias=eps_sb[:])
            rstd = rows.tile([1, TOK], bf16, tag="rowb")
            with nc.allow_low_precision(reason="bf16 rstd broadcast"):
                nc.scalar.activation(rstd, lnv, AF.Exp, scale=-0.5)
            mub16 = rows.tile([1, TOK], bf16, tag="rowb")
            with nc.allow_low_precision(reason="bf16 mu broadcast"):
                nc.vector.tensor_copy(mub16, muf)
            mub = psum.tile([P, TOK], f32, tag="big", bufs=2, name="l2mub")
            nc.tensor.matmul(mub, ones_rb, mub16, start=True, stop=True)
            rsb = psum.tile([P, TOK], f32, tag="big", bufs=2, name="l2rsb")
            nc.tensor.matmul(rsb, ones_rb, rstd, start=True, stop=True)
            mubs = lnt.tile([P, TOK], bf16, tag="l2mb")
            nc.vector.tensor_copy(mubs, mub)
            rsbs = lnt.tile([P, TOK], bf16, tag="l2rb")
            nc.vector.tensor_copy(rsbs, rsb)
            for k in range(KD):
                t1 = lnt.tile([P, TOK], bf16, tag="ln2")
                nc.vector.tensor_tensor(out=t1, in0=h1T[:, k, :], in1=mubs,
                                        op=OP.subtract)
                nc.vector.tensor_tensor(out=mT[:, k, :], in0=t1, in1=rsbs,
                                        op=OP.mult)

        # ---- phase 7: MLP ----
        hT = mlp.tile([P, KDI, TOK], bf16)
        with nc.named_scope("fc1"):
            for j in range(KDI):
                fwt = wgt.tile([P, KD, P], bf16, tag="fw", bufs=3)
                nc.sync.dma_start(fwt, fw[j])
                ps = psum.tile([P, TOK], f32, tag="big", bufs=2)
                for k in range(KD):
                    nc.tensor.matmul(ps, fwt[:, k, :], mT[:, k, :],
                                     start=(k == 0), stop=(k == KD - 1))
                nc.scalar.activation(hT[:, j, :], ps, AF.Gelu_apprx_tanh,
                                     bias=fb_sb[:, j:j + 1])
        with nc.named_scope("fc2"):
            for f in range(KD):
                gwt = wgt.tile([P, KDI, P], bf16, tag="gw", bufs=2)
                nc.sync.dma_start(gwt, gw[f])
                ps = psum.tile([P, TOK], f32, tag="big", bufs=2)
                for k in range(KDI):
                    nc.tensor.matmul(ps, gwt[:, k, :], hT[:, k, :],
                                     start=(k == 0), stop=(k == KDI - 1))
                o = outp.tile([P, TOK], f32, tag="ot")
                nc.vector.tensor_scalar_add(o, ps, gb_sb[:, f:f + 1])
                nc.vector.tensor_tensor(out=o, in0=o, in1=h1T[:, f, :],
                                        op=OP.add)
                nc.sync.dma_start(outT[P * f:P * (f + 1), :], o)

    nc.compile()
    return nc


def shard_inputs(inputs):
    """Full inputs -> list of 8 per-core input dicts (host-side layout only)."""
    bf16 = ml_dtypes.bfloat16
    f32 = np.float32
    hs = np.asarray(inputs["hidden_states"], f32).reshape(TT, D)
    l1g = np.asarray(inputs["ln1_g"], f32)
    l1b = np.asarray(inputs["ln1_b"], f32)
    l2g = np.asarray(inputs["ln2_g"], f32)
    l2b = np.asarray(inputs["ln2_b"], f32)
    # fold LN1 gamma/beta into attn_w/attn_b, LN2 into fc_w/fc_b
    attn_w = np.asarray(inputs["attn_w"], f32) * l1g[:, None]
    attn_b = np.asarray(inputs["attn_b"], f32) + l1b @ np.asarray(inputs["attn_w"], f32)
    fc_w = np.asarray(inputs["fc_w"], f32) * l2g[:, None]
    fc_b = np.asarray(inputs["fc_b"], f32) + l2b @ np.asarray(inputs["fc_w"], f32)

    def col(v):  # [D] -> [P, KD]
        return np.ascontiguousarray(np.asarray(v, f32).reshape(KD, P).T)

    pw = np.ascontiguousarray(np.asarray(inputs["proj_w"], f32)
                              .reshape(KD, P, KD, P).transpose(2, 1, 0, 3)
                              .astype(bf16))
    fw = np.ascontiguousarray(fc_w.reshape(KD, P, KDI, P).transpose(2, 1, 0, 3)
                              .astype(bf16))
    gw = np.ascontiguousarray(np.asarray(inputs["fc2_w"], f32)
                              .reshape(KDI, P, KD, P).transpose(2, 1, 0, 3)
                              .astype(bf16))
    pb = col(inputs["proj_b"])
    fbv = np.ascontiguousarray(fc_b.reshape(KDI, P).T)
    gbv = col(inputs["fc2_b"])

    ii, jj = np.meshgrid(np.arange(P), np.arange(QCH), indexing="ij")
    mkv = np.stack([(jj >= ii), (jj >= ii + P)]).astype(bf16)

    maps = []
    for c in range(NCORES):
        cols = np.r_[P * c:P * (c + 1),
                     D + P * c:D + P * (c + 1),
                     2 * D + P * c:2 * D + P * (c + 1)]
        aw_c = np.ascontiguousarray(
            attn_w[:, cols].reshape(KD, P, 3 * P).transpose(1, 0, 2)
            .astype(bf16))
        ab_c = np.ascontiguousarray(attn_b[cols], dtype=f32)
        xT_c = np.ascontiguousarray(
            hs[TOK * c:TOK * (c + 1)].T.reshape(KD, P, TOK).transpose(1, 0, 2)
        ).astype(bf16)
        maps.append({
            "xT": xT_c, "aw": aw_c, "ab": ab_c,
            "pw": pw, "pb": pb, "fw": fw, "fb": fbv, "gw": gw, "gb": gbv,
            "mk": mkv,
        })
    return maps


def unshard(results):
    out = np.concatenate([np.asarray(r["outT"]).T for r in results], axis=0)
    return np.ascontiguousarray(out.reshape(B, S, D))


def kernel(**inputs):
    global _CACHED_NC
    from concourse.bass_utils import run_bass_kernel_spmd
    if _CACHED_NC is None:
        _CACHED_NC = build_nc()
    in_maps = shard_inputs(inputs)
    res = run_bass_kernel_spmd(_CACHED_NC, in_maps,
                               core_ids=list(range(NCORES)))
    return unshard(res.results)
